# revision 15
# baseline (speedup 1.0000x reference)
"""GridTransformerBlock TRN2 kernel (v2).

Sharding: batch-parallel over B=8 -> one batch per NeuronCore, zero collectives.

Layout: the reference's (B,S,E)->(B,E,H,W) reshape is a raw reinterpret, so per
batch the buffer is 256 channel planes of 128x128; each 16x16 window's
attention tile T is [tokens=channels, features=window pixels].

v2 structure (vs v1):
- bf16 operands everywhere (PSUM accumulation stays fp32); host pre-casts.
- Zero attention biases let us fold weights on the host:
    scores = T (Wq Wk^T / sqrt(E)) T^T   -> one fused M matrix, no K tensor
    y      = A (T (Wv Wo))               -> one fused N matrix, no separate V/O
- Window-pair batching: QKV-side projections run at N=512 moving dim.
- Phase split: attention for all 8 stripes (Exp table only), then FFN for all
  stripes (Gelu table only) -> 2 ACT table loads instead of ~44, and dense
  back-to-back PE streams that keep the HAM clock-gate warm.
- ys (attention output / residual) kept resident in SBUF in bf16 (8 MB).
- Multi-group bn_stats, batched 2-iteration Newton rsqrt, PSUM evacuations
  balanced across Vector and Scalar engines.
"""

import os
import sys
import numpy as np

for _p in ("/opt/trn_rl_repo", "/root/.axon_site/_ro/trn_rl_repo"):
    if _p not in sys.path and os.path.isdir(_p):
        sys.path.insert(0, _p)

B, S, E, FF = 8, 16384, 256, 1024
H, W, G = 128, 128, 16
Hn, Wn = 8, 8

_CACHE = {}


def _build_fast(ffn_flags):
    """Fused-attention build: requires zero attention biases (bq/bk/bv/bo)."""
    use_b1, use_b2, use_g1, use_g2 = ffn_flags
    import concourse.bacc as bacc
    import concourse.mybir as mybir
    import concourse.tile as tile
    from contextlib import ExitStack

    F32 = mybir.dt.float32
    BF16 = mybir.dt.bfloat16
    I32 = mybir.dt.int32
    AF = mybir.ActivationFunctionType
    OP = mybir.AluOpType

    nc = bacc.Bacc("TRN2", target_bir_lowering=False, debug=False, num_devices=8)

    x_d = nc.dram_tensor("x", [S, E], BF16, kind="ExternalInput")
    m_d = nc.dram_tensor("m", [E, E], BF16, kind="ExternalInput")   # Wq Wk^T / 16
    n_d = nc.dram_tensor("n", [E, E], BF16, kind="ExternalInput")   # Wv Wo
    w1_d = nc.dram_tensor("w1", [E, FF], BF16, kind="ExternalInput")
    w2_d = nc.dram_tensor("w2", [FF, E], BF16, kind="ExternalInput")
    id_d = nc.dram_tensor("ident", [128, 128], BF16, kind="ExternalInput")
    out_d = nc.dram_tensor("out", [S, E], F32, kind="ExternalOutput")
    if use_b1:
        b1_d = nc.dram_tensor("b1", [FF], F32, kind="ExternalInput")
    if use_b2:
        b2_d = nc.dram_tensor("b2", [E], F32, kind="ExternalInput")
    if use_g1:
        g1_d = nc.dram_tensor("g1", [E], F32, kind="ExternalInput")
        be1_d = nc.dram_tensor("be1", [E], F32, kind="ExternalInput")
    if use_g2:
        g2_d = nc.dram_tensor("g2", [E], F32, kind="ExternalInput")
        be2_d = nc.dram_tensor("be2", [E], F32, kind="ExternalInput")

    import concourse.bass as bass

    def bcast_ap(dram, n=256):
        return bass.AP(tensor=dram.ap().tensor, offset=0, ap=[[0, 128], [1, n]])

    X = x_d.ap().rearrange("(c t) e -> c (t e)", t=64)      # [256, 16384]
    OUTV = out_d.ap().rearrange("(c t) e -> c t e", t=64)   # [256, 64, 256]

    with tile.TileContext(nc) as tc:
        with ExitStack() as ctx:
            const = ctx.enter_context(tc.tile_pool(name="const", bufs=1))
            xsp = ctx.enter_context(tc.tile_pool(name="xsp", bufs=3))
            ysp = ctx.enter_context(tc.tile_pool(name="ysp", bufs=16))
            twp = ctx.enter_context(tc.tile_pool(name="twp", bufs=2))
            att = ctx.enter_context(tc.tile_pool(name="att", bufs=2))
            stp = ctx.enter_context(tc.tile_pool(name="stp", bufs=2))
            ffn = ctx.enter_context(tc.tile_pool(name="ffn", bufs=2))
            lnp = ctx.enter_context(tc.tile_pool(name="lnp", bufs=2))
            msc = ctx.enter_context(tc.tile_pool(name="msc", bufs=2))
            outp = ctx.enter_context(tc.tile_pool(name="outp", bufs=4))
            p2 = ctx.enter_context(tc.tile_pool(name="p2", bufs=3, space="PSUM"))
            p1 = ctx.enter_context(tc.tile_pool(name="p1", bufs=2, space="PSUM"))

            ident = const.tile([128, 128], BF16)
            nc.sync.dma_start(out=ident, in_=id_d.ap()[:, :])
            m_t = const.tile([128, 2, 256], BF16)
            nc.sync.dma_start(out=m_t, in_=m_d.ap().rearrange("(ph k) g -> k ph g", k=128))
            n_t = const.tile([128, 2, 256], BF16)
            nc.sync.dma_start(out=n_t, in_=n_d.ap().rearrange("(ph k) g -> k ph g", k=128))
            w1_t = const.tile([128, 2, 1024], BF16)
            nc.sync.dma_start(out=w1_t, in_=w1_d.ap().rearrange("(eh k) f -> k eh f", k=128))
            w2_t = const.tile([128, 8, 256], BF16)
            nc.sync.dma_start(out=w2_t, in_=w2_d.ap().rearrange("(fm k) e -> k fm e", k=128))
            if use_b1:
                b1_t = const.tile([128, 8], F32)
                nc.sync.dma_start(out=b1_t, in_=b1_d.ap().rearrange("(fm p) -> p fm", p=128))
            if use_b2:
                b2_t = const.tile([128, 2], F32)
                nc.sync.dma_start(out=b2_t, in_=b2_d.ap().rearrange("(em p) -> p em", p=128))
            if use_g1:
                g1_bc = const.tile([128, 256], F32)
                nc.sync.dma_start(out=g1_bc, in_=bcast_ap(g1_d))
                be1_bc = const.tile([128, 256], F32)
                nc.sync.dma_start(out=be1_bc, in_=bcast_ap(be1_d))
            if use_g2:
                g2_bc = const.tile([128, 256], F32)
                nc.sync.dma_start(out=g2_bc, in_=bcast_ap(g2_d))
                be2_bc = const.tile([128, 256], F32)
                nc.sync.dma_start(out=be2_bc, in_=bcast_ap(be2_d))

            def newton_rsqrt(var_ap, n, tag):
                """rstd = 1/sqrt(var + eps) for a [128, n] strided var AP."""
                w = msc.tile([128, n], F32, tag=f"nw_w{tag}")
                nc.vector.tensor_scalar(out=w, in0=var_ap, scalar1=1e-5,
                                        scalar2=None, op0=OP.add)
                r = msc.tile([128, n], F32, tag=f"nw_r{tag}")
                nc.vector.tensor_scalar(out=r.bitcast(I32), in0=w.bitcast(I32),
                                        scalar1=1, scalar2=None,
                                        op0=OP.logical_shift_right)
                nc.vector.tensor_scalar(out=r.bitcast(I32), in0=r.bitcast(I32),
                                        scalar1=0xFFFFFFFF, scalar2=None,
                                        op0=OP.bitwise_xor)
                nc.vector.tensor_scalar(out=r.bitcast(I32), in0=r.bitcast(I32),
                                        scalar1=0x5F375A86 + 1, scalar2=None,
                                        op0=OP.add)
                rsq = msc.tile([128, n], F32, tag=f"nw_rsq{tag}")
                u = msc.tile([128, n], F32, tag=f"nw_u{tag}")
                v = msc.tile([128, n], F32, tag=f"nw_v{tag}")
                for _ in range(2):
                    nc.vector.tensor_mul(rsq, r, r)
                    nc.vector.tensor_mul(u, rsq, w)
                    nc.vector.tensor_scalar(out=v, in0=u, scalar1=-0.5, scalar2=1.5,
                                            op0=OP.mult, op1=OP.add)
                    nc.vector.tensor_mul(r, r, v)
                return r

            ys_all = []  # [hn][ct] -> [128, 2048] bf16 tile

            # ================= Phase A: attention, all stripes =================
            for hn in range(Hn):
                xs_pair = []
                for ct in range(2):
                    t = xsp.tile([128, 2048], BF16, tag="xs")
                    nc.sync.dma_start(
                        out=t, in_=X[ct * 128:(ct + 1) * 128, hn * 2048:(hn + 1) * 2048])
                    xs_pair.append(t)
                ys_pair = [ysp.tile([128, 2048], BF16, tag="ys", name=f"ys{hn}_{i}")
                           for i in range(2)]
                ys_all.append(ys_pair)

                for pk in range(4):  # window pair (2pk, 2pk+1)
                    # gather: per-window contiguous [128ch, 256px] blocks
                    t_sb = twp.tile([128, 2, 2, 256], BF16, tag="tw")
                    for ct in range(2):
                        xv = xs_pair[ct][:, :].rearrange("p (g1 w) -> p g1 w", w=128)
                        for wi in range(2):
                            wn = pk * 2 + wi
                            nc.gpsimd.tensor_copy(
                                t_sb[:, ct, wi, :].rearrange(
                                    "p (g1 g2) -> p g1 g2", g2=16),
                                xv[:, :, wn * 16:(wn + 1) * 16])

                    # tt = T^T per window via DMA-xbar: [px(ph), wi, ch(2ct*128)]
                    tt = att.tile([128, 2, 2, 256], BF16, tag="tt")
                    for ct in range(2):
                        for wi in range(2):
                            for ph in range(2):
                                nc.sync.dma_start_transpose(
                                    out=tt[:, ph, wi, ct * 128:(ct + 1) * 128],
                                    in_=t_sb[:, ct, wi, ph * 128:(ph + 1) * 128])

                    # u^T = M^T T^T : [g(2 chunks gh), (wi, tok)]
                    u_ps = p2.tile([128, 2, 512], F32, tag="p2")
                    for gh in range(2):
                        for ph in range(2):
                            nc.tensor.matmul(u_ps[:, gh, :],
                                             lhsT=m_t[:, ph, gh * 128:(gh + 1) * 128],
                                             rhs=tt[:, ph, :, :],
                                             start=ph == 0, stop=ph == 1)
                    ut = att.tile([128, 2, 512], BF16, tag="ut")
                    nc.vector.tensor_copy(ut, u_ps)

                    # u2 = T N : [(wi, tok-chunk ct), fo]
                    u2_ps = p2.tile([128, 2, 2, 256], F32, tag="p2")
                    for wi in range(2):
                        for ct in range(2):
                            for ph in range(2):
                                nc.tensor.matmul(
                                    u2_ps[:, wi, ct, :],
                                    lhsT=tt[:, ph, wi, ct * 128:(ct + 1) * 128],
                                    rhs=n_t[:, ph, :],
                                    start=ph == 0, stop=ph == 1)
                    u2 = att.tile([128, 2, 2, 256], BF16, tag="u2")
                    nc.vector.tensor_copy(u2, u2_ps)

                    # scores + exp (unnormalized), denominator accumulated
                    den = stp.tile([128, 4], F32, tag="den")
                    aa = att.tile([128, 2, 2, 256], BF16, tag="aa")
                    for wi in range(2):
                        s_ps = p1.tile([128, 2, 256], F32, tag="p1")
                        for th in range(2):
                            for gh in range(2):
                                nc.tensor.matmul(
                                    s_ps[:, th, :],
                                    lhsT=ut[:, gh, wi * 256 + th * 128:
                                            wi * 256 + (th + 1) * 128],
                                    rhs=tt[:, gh, wi, :],
                                    start=gh == 0, stop=gh == 1)
                        for th in range(2):
                            nc.scalar.activation(
                                out=aa[:, wi, th, :], in_=s_ps[:, th, :],
                                func=AF.Exp,
                                accum_out=den[:, wi * 2 + th:wi * 2 + th + 1])
                    rec = stp.tile([128, 4], F32, tag="rec")
                    nc.vector.reciprocal(rec, den)

                    # at = A^T (unnormalized) via DMA-xbar: [(wi), k-chunk t2h, q]
                    at = att.tile([128, 2, 2, 256], BF16, tag="at")
                    for wi in range(2):
                        for th in range(2):
                            for t2h in range(2):
                                nc.sync.dma_start_transpose(
                                    out=at[:, wi, t2h, th * 128:(th + 1) * 128],
                                    in_=aa[:, wi, th, t2h * 128:(t2h + 1) * 128])

                    # y_window = A u2, scaled by 1/den at evacuation
                    for wi in range(2):
                        o_ps = p1.tile([128, 2, 256], F32, tag="p1")
                        for th in range(2):
                            for t2h in range(2):
                                nc.tensor.matmul(
                                    o_ps[:, th, :],
                                    lhsT=at[:, wi, t2h, th * 128:(th + 1) * 128],
                                    rhs=u2[:, wi, t2h, :],
                                    start=t2h == 0, stop=t2h == 1)
                        wn = pk * 2 + wi
                        for th in range(2):
                            ys_sl = ys_pair[th][:, :].rearrange(
                                "p (g1 w) -> p g1 w", w=128)[:, :, wn * 16:(wn + 1) * 16]
                            nc.vector.tensor_scalar(
                                out=ys_sl,
                                in0=o_ps[:, th, :].rearrange("p (a b) -> p a b", b=16),
                                scalar1=rec[:, wi * 2 + th:wi * 2 + th + 1],
                                scalar2=None, op0=OP.mult)

            # ================= Phase B: FFN + LNs, all stripes =================
            for hn in range(Hn):
                ys_pair = ys_all[hn]
                for nb in range(4):
                    chunks = [(q // 8, q % 8) for q in range(nb * 4, nb * 4 + 4)]
                    yt = ffn.tile([128, 2, 512], BF16, tag="yt")
                    for eh in range(2):
                        for pos, (ct, j) in enumerate(chunks):
                            nc.sync.dma_start_transpose(
                                out=yt[:, eh, pos * 128:(pos + 1) * 128],
                                in_=ys_pair[ct][:, j * 256 + eh * 128:
                                                j * 256 + (eh + 1) * 128])

                    hh = ffn.tile([128, 8, 512], BF16, tag="hh")
                    for fp in range(4):
                        h_ps = p2.tile([128, 2, 512], F32, tag="p2")
                        for i in range(2):
                            fm = fp * 2 + i
                            for eh in range(2):
                                nc.tensor.matmul(h_ps[:, i, :],
                                                 lhsT=w1_t[:, eh, fm * 128:(fm + 1) * 128],
                                                 rhs=yt[:, eh, :],
                                                 start=eh == 0, stop=eh == 1)
                        if use_b1:
                            for i in range(2):
                                fm = fp * 2 + i
                                nc.scalar.activation(out=hh[:, fm, :], in_=h_ps[:, i, :],
                                                     func=AF.Gelu,
                                                     bias=b1_t[:, fm:fm + 1])
                        else:
                            nc.scalar.activation(out=hh[:, fp * 2:(fp + 1) * 2, :],
                                                 in_=h_ps, func=AF.Gelu)

                    ft = ffn.tile([128, 2, 512], BF16, tag="ft")
                    for em in range(2):
                        f_ps = p1.tile([128, 512], F32, tag="p1")
                        for fm in range(8):
                            nc.tensor.matmul(f_ps,
                                             lhsT=w2_t[:, fm, em * 128:(em + 1) * 128],
                                             rhs=hh[:, fm, :],
                                             start=fm == 0, stop=fm == 7)
                        if use_b2:
                            nc.scalar.activation(out=ft[:, em, :], in_=f_ps,
                                                 func=AF.Identity,
                                                 bias=b2_t[:, em:em + 1])
                        else:
                            nc.scalar.activation(out=ft[:, em, :], in_=f_ps,
                                                 func=AF.Copy)

                    zt = lnp.tile([128, 4, 256], BF16, tag="zt")
                    for pos in range(4):
                        for em in range(2):
                            nc.sync.dma_start_transpose(
                                out=zt[:, pos, em * 128:(em + 1) * 128],
                                in_=ft[:, em, pos * 128:(pos + 1) * 128])

                    bst1 = msc.tile([128, 4, 6], F32, tag="bst1")
                    for pos in range(4):
                        nc.vector.bn_stats(out=bst1[:, pos, :], in_=zt[:, pos, :])
                    mvs1 = msc.tile([128, 4, 2], F32, tag="mvs1")
                    for pos in range(4):
                        nc.vector.bn_aggr(out=mvs1[:, pos, :], in_=bst1[:, pos, :])
                    rs1 = newton_rsqrt(mvs1[:, :, 1], 4, "a")

                    y2 = lnp.tile([128, 4, 256], BF16, tag="y2")
                    for pos, (ct, j) in enumerate(chunks):
                        ln1 = lnp.tile([128, 256], BF16, tag="ln1")
                        nc.vector.tensor_scalar(
                            out=ln1, in0=zt[:, pos, :],
                            scalar1=mvs1[:, pos, 0:1], scalar2=rs1[:, pos:pos + 1],
                            op0=OP.subtract, op1=OP.mult)
                        if use_g1:
                            nc.gpsimd.tensor_mul(ln1, ln1, g1_bc)
                            nc.gpsimd.tensor_add(ln1, ln1, be1_bc)
                        nc.gpsimd.tensor_add(
                            y2[:, pos, :], ln1,
                            ys_pair[ct][:, j * 256:(j + 1) * 256])

                    bst2 = msc.tile([128, 4, 6], F32, tag="bst2")
                    for pos in range(4):
                        nc.vector.bn_stats(out=bst2[:, pos, :], in_=y2[:, pos, :])
                    mvs2 = msc.tile([128, 4, 2], F32, tag="mvs2")
                    for pos in range(4):
                        nc.vector.bn_aggr(out=mvs2[:, pos, :], in_=bst2[:, pos, :])
                    rs2 = newton_rsqrt(mvs2[:, :, 1], 4, "b")

                    for pos, (ct, j) in enumerate(chunks):
                        ln2 = lnp.tile([128, 256], BF16, tag="ln2")
                        nc.gpsimd.tensor_scalar(
                            out=ln2, in0=y2[:, pos, :],
                            scalar1=mvs2[:, pos, 0:1], scalar2=rs2[:, pos:pos + 1],
                            op0=OP.subtract, op1=OP.mult)
                        if use_g2:
                            nc.gpsimd.tensor_mul(ln2, ln2, g2_bc)
                            nc.gpsimd.tensor_add(ln2, ln2, be2_bc)
                        outt = outp.tile([128, 256], F32, tag="outt")
                        nc.gpsimd.tensor_add(outt, ln2, y2[:, pos, :])
                        nc.scalar.dma_start(
                            out=OUTV[ct * 128:(ct + 1) * 128, hn * 8 + j, :],
                            in_=outt)

    nc.compile()
    return nc


def _round_f32r(x):
    u = np.ascontiguousarray(x, np.float32).view(np.uint32)
    return ((u + np.uint32(0x800)) & np.uint32(0xFFFFF000)).view(np.float32)


def _build_v1(flags):
    """Unfused fallback (handles attention biases); f32r, per-window."""
    use_bqk, use_bv, use_bo, use_b1, use_b2, use_g1, use_g2 = flags
    import concourse.bacc as bacc
    import concourse.mybir as mybir
    import concourse.tile as tile
    from contextlib import ExitStack

    F32 = mybir.dt.float32
    F32R = mybir.dt.float32r
    I32 = mybir.dt.int32
    AF = mybir.ActivationFunctionType
    OP = mybir.AluOpType

    nc = bacc.Bacc("TRN2", target_bir_lowering=False, debug=False, num_devices=8)

    x_d = nc.dram_tensor("x", [S, E], F32R, kind="ExternalInput")
    wq_d = nc.dram_tensor("wq", [E, E], F32R, kind="ExternalInput")
    wk_d = nc.dram_tensor("wk", [E, E], F32R, kind="ExternalInput")
    wv_d = nc.dram_tensor("wv", [E, E], F32R, kind="ExternalInput")
    wo_d = nc.dram_tensor("wo", [E, E], F32R, kind="ExternalInput")
    w1_d = nc.dram_tensor("w1", [E, FF], F32R, kind="ExternalInput")
    w2_d = nc.dram_tensor("w2", [FF, E], F32R, kind="ExternalInput")
    id_d = nc.dram_tensor("ident", [128, 128], F32R, kind="ExternalInput")
    out_d = nc.dram_tensor("out", [S, E], F32, kind="ExternalOutput")
    if use_bqk:
        bq_d = nc.dram_tensor("bq", [E], F32, kind="ExternalInput")
        bk_d = nc.dram_tensor("bk", [E], F32, kind="ExternalInput")
    if use_bv:
        bv_d = nc.dram_tensor("bv", [E], F32, kind="ExternalInput")
    if use_bo:
        bo_d = nc.dram_tensor("bo", [E], F32, kind="ExternalInput")
    if use_b1:
        b1_d = nc.dram_tensor("b1", [FF], F32, kind="ExternalInput")
    if use_b2:
        b2_d = nc.dram_tensor("b2", [E], F32, kind="ExternalInput")
    if use_g1:
        g1_d = nc.dram_tensor("g1", [E], F32, kind="ExternalInput")
        be1_d = nc.dram_tensor("be1", [E], F32, kind="ExternalInput")
    if use_g2:
        g2_d = nc.dram_tensor("g2", [E], F32, kind="ExternalInput")
        be2_d = nc.dram_tensor("be2", [E], F32, kind="ExternalInput")

    import concourse.bass as bass

    def bcast_ap(dram, n=256):
        return bass.AP(tensor=dram.ap().tensor, offset=0, ap=[[0, 128], [1, n]])

    X = x_d.ap().rearrange("(c t) e -> c (t e)", t=64)
    OUTV = out_d.ap().rearrange("(c t) e -> c t e", t=64)

    with tile.TileContext(nc) as tc:
        with ExitStack() as ctx:
            const = ctx.enter_context(tc.tile_pool(name="const", bufs=1))
            xsp = ctx.enter_context(tc.tile_pool(name="xsp", bufs=4))
            ysp = ctx.enter_context(tc.tile_pool(name="ysp", bufs=4))
            twp = ctx.enter_context(tc.tile_pool(name="twp", bufs=2))
            att = ctx.enter_context(tc.tile_pool(name="att", bufs=2))
            stp = ctx.enter_context(tc.tile_pool(name="stp", bufs=4))
            ffn = ctx.enter_context(tc.tile_pool(name="ffn", bufs=2))
            lnp = ctx.enter_context(tc.tile_pool(name="lnp", bufs=4))
            msc = ctx.enter_context(tc.tile_pool(name="msc", bufs=4))
            pA = ctx.enter_context(tc.tile_pool(name="pA", bufs=3, space="PSUM"))
            pH = ctx.enter_context(tc.tile_pool(name="pH", bufs=1, space="PSUM"))
            pF = ctx.enter_context(tc.tile_pool(name="pF", bufs=3, space="PSUM"))

            ident = const.tile([128, 128], F32R)
            nc.sync.dma_start(out=ident, in_=id_d.ap()[:, :])
            wq_t = const.tile([128, 2, 256], F32R)
            wk_t = const.tile([128, 2, 256], F32R)
            wv_t = const.tile([128, 2, 256], F32R)
            wo_t = const.tile([128, 2, 256], F32R)
            for t, d in ((wq_t, wq_d), (wk_t, wk_d), (wv_t, wv_d), (wo_t, wo_d)):
                nc.sync.dma_start(out=t, in_=d.ap().rearrange("(eh k) f -> k eh f", k=128))
            w1_t = const.tile([128, 2, 1024], F32R)
            nc.sync.dma_start(out=w1_t, in_=w1_d.ap().rearrange("(eh k) f -> k eh f", k=128))
            w2_t = const.tile([128, 8, 256], F32R)
            nc.sync.dma_start(out=w2_t, in_=w2_d.ap().rearrange("(fm k) e -> k fm e", k=128))
            if use_bqk:
                bq_t = const.tile([128, 2], F32)
                nc.sync.dma_start(out=bq_t, in_=bq_d.ap().rearrange("(fh p) -> p fh", p=128))
                bk_t = const.tile([128, 2], F32)
                nc.sync.dma_start(out=bk_t, in_=bk_d.ap().rearrange("(fh p) -> p fh", p=128))
            if use_bv:
                bv_bc = const.tile([128, 2, 256], F32)
                nc.sync.dma_start(
                    out=bv_bc,
                    in_=bass.AP(tensor=bv_d.ap().tensor, offset=0,
                                ap=[[0, 128], [0, 2], [1, 256]]))
            if use_bo:
                bo_st = const.tile([128, 2048], F32)
                nc.sync.dma_start(
                    out=bo_st.rearrange("p (g1 wn g2) -> p g1 wn g2", wn=8, g2=16),
                    in_=bass.AP(tensor=bo_d.ap().tensor, offset=0,
                                ap=[[0, 128], [16, 16], [0, 8], [1, 16]]))
            if use_b1:
                b1_t = const.tile([128, 8], F32)
                nc.sync.dma_start(out=b1_t, in_=b1_d.ap().rearrange("(fm p) -> p fm", p=128))
            if use_b2:
                b2_t = const.tile([128, 2], F32)
                nc.sync.dma_start(out=b2_t, in_=b2_d.ap().rearrange("(em p) -> p em", p=128))
            if use_g1:
                g1_bc = const.tile([128, 256], F32)
                nc.sync.dma_start(out=g1_bc, in_=bcast_ap(g1_d))
                be1_bc = const.tile([128, 256], F32)
                nc.sync.dma_start(out=be1_bc, in_=bcast_ap(be1_d))
            if use_g2:
                g2_bc = const.tile([128, 256], F32)
                nc.sync.dma_start(out=g2_bc, in_=bcast_ap(g2_d))
                be2_bc = const.tile([128, 256], F32)
                nc.sync.dma_start(out=be2_bc, in_=bcast_ap(be2_d))

            def newton_rsqrt(var_ap, n):
                w = msc.tile([128, n], F32, tag="nw_w")
                nc.vector.tensor_scalar(out=w, in0=var_ap, scalar1=1e-5,
                                        scalar2=None, op0=OP.add)
                r = msc.tile([128, n], F32, tag="nw_r")
                nc.vector.tensor_scalar(out=r.bitcast(I32), in0=w.bitcast(I32),
                                        scalar1=1, scalar2=None,
                                        op0=OP.logical_shift_right)
                nc.vector.tensor_scalar(out=r.bitcast(I32), in0=r.bitcast(I32),
                                        scalar1=0xFFFFFFFF, scalar2=None,
                                        op0=OP.bitwise_xor)
                nc.vector.tensor_scalar(out=r.bitcast(I32), in0=r.bitcast(I32),
                                        scalar1=0x5F375A86 + 1, scalar2=None,
                                        op0=OP.add)
                rsq = msc.tile([128, n], F32, tag="nw_rsq")
                u = msc.tile([128, n], F32, tag="nw_u")
                v = msc.tile([128, n], F32, tag="nw_v")
                for _ in range(3):
                    nc.vector.tensor_mul(rsq, r, r)
                    nc.vector.tensor_mul(u, rsq, w)
                    nc.vector.tensor_scalar(out=v, in0=u, scalar1=-0.5, scalar2=1.5,
                                            op0=OP.mult, op1=OP.add)
                    nc.vector.tensor_mul(r, r, v)
                return r

            for hn in range(Hn):
                xs_pair = []
                for ct in range(2):
                    t = xsp.tile([128, 2048], F32R, tag="xs")
                    nc.sync.dma_start(
                        out=t, in_=X[ct * 128:(ct + 1) * 128, hn * 2048:(hn + 1) * 2048])
                    xs_pair.append(t)
                ys_pair = [ysp.tile([128, 2048], F32R, tag="ys", name=f"ys{hn}_{i}")
                           for i in range(2)]

                for wn in range(Wn):
                    t_sb = twp.tile([128, 2, 256], F32R, tag="tw")
                    for ct in range(2):
                        xv = xs_pair[ct][:, :].rearrange("p (g1 w) -> p g1 w", w=128)
                        nc.gpsimd.tensor_copy(
                            t_sb[:, ct, :].rearrange("p (g1 g2) -> p g1 g2", g2=16),
                            xv[:, :, wn * 16:(wn + 1) * 16])
                    tt_ps = pA.tile([128, 2, 256], F32, tag="pA")
                    for eh in range(2):
                        for ct in range(2):
                            nc.tensor.transpose(
                                tt_ps[:, eh, ct * 128:(ct + 1) * 128].bitcast(F32R),
                                t_sb[:, ct, eh * 128:(eh + 1) * 128], ident)
                    tt = att.tile([128, 2, 256], F32R, tag="tt")
                    nc.vector.tensor_copy(tt, tt_ps)

                    qt_ps = pA.tile([128, 2, 256], F32, tag="pA")
                    for fh in range(2):
                        for eh in range(2):
                            nc.tensor.matmul(qt_ps[:, fh, :],
                                             lhsT=wq_t[:, eh, fh * 128:(fh + 1) * 128],
                                             rhs=tt[:, eh, :],
                                             start=eh == 0, stop=eh == 1)
                    qt = att.tile([128, 2, 256], F32R, tag="qt")
                    if use_bqk:
                        for fh in range(2):
                            nc.scalar.activation(out=qt[:, fh, :], in_=qt_ps[:, fh, :],
                                                 func=AF.Identity,
                                                 bias=bq_t[:, fh:fh + 1])
                    else:
                        nc.vector.tensor_copy(qt, qt_ps)

                    kt_ps = pA.tile([128, 2, 256], F32, tag="pA")
                    for fh in range(2):
                        for eh in range(2):
                            nc.tensor.matmul(kt_ps[:, fh, :],
                                             lhsT=wk_t[:, eh, fh * 128:(fh + 1) * 128],
                                             rhs=tt[:, eh, :],
                                             start=eh == 0, stop=eh == 1)
                    kt = att.tile([128, 2, 256], F32R, tag="kt")
                    if use_bqk:
                        for fh in range(2):
                            nc.scalar.activation(out=kt[:, fh, :], in_=kt_ps[:, fh, :],
                                                 func=AF.Identity,
                                                 bias=bk_t[:, fh:fh + 1])
                    else:
                        nc.vector.tensor_copy(kt, kt_ps)

                    v_ps = pA.tile([128, 2, 256], F32, tag="pA")
                    for ch in range(2):
                        for eh in range(2):
                            nc.tensor.matmul(v_ps[:, ch, :],
                                             lhsT=tt[:, eh, ch * 128:(ch + 1) * 128],
                                             rhs=wv_t[:, eh, :],
                                             start=eh == 0, stop=eh == 1)
                    vv = att.tile([128, 2, 256], F32R, tag="vv")
                    if use_bv:
                        nc.vector.tensor_add(vv, v_ps, bv_bc)
                    else:
                        nc.scalar.activation(out=vv, in_=v_ps, func=AF.Copy)

                    s_ps = pA.tile([128, 2, 256], F32, tag="pA")
                    for th in range(2):
                        for fh in range(2):
                            nc.tensor.matmul(s_ps[:, th, :],
                                             lhsT=qt[:, fh, th * 128:(th + 1) * 128],
                                             rhs=kt[:, fh, :],
                                             start=fh == 0, stop=fh == 1)
                    aa = att.tile([128, 2, 256], F32R, tag="aa")
                    den = stp.tile([128, 2], F32, tag="den")
                    for th in range(2):
                        nc.scalar.activation(out=aa[:, th, :], in_=s_ps[:, th, :],
                                             func=AF.Exp,
                                             accum_out=den[:, th:th + 1])
                    rec = stp.tile([128, 2], F32, tag="rec")
                    nc.vector.reciprocal(rec, den)

                    at_ps = pA.tile([128, 2, 256], F32, tag="pA")
                    for t2h in range(2):
                        for th in range(2):
                            nc.tensor.transpose(
                                at_ps[:, t2h, th * 128:(th + 1) * 128].bitcast(F32R),
                                aa[:, th, t2h * 128:(t2h + 1) * 128], ident)
                    at = att.tile([128, 2, 256], F32R, tag="at")
                    nc.scalar.activation(out=at, in_=at_ps, func=AF.Copy)

                    ot_ps = pA.tile([128, 2, 256], F32, tag="pA")
                    for fh in range(2):
                        for t2h in range(2):
                            nc.tensor.matmul(ot_ps[:, fh, :],
                                             lhsT=vv[:, t2h, fh * 128:(fh + 1) * 128],
                                             rhs=at[:, t2h, :],
                                             start=t2h == 0, stop=t2h == 1)
                    ot = att.tile([128, 2, 256], F32R, tag="ot")
                    nc.scalar.activation(out=ot, in_=ot_ps, func=AF.Copy)

                    o2_ps = pA.tile([128, 2, 256], F32, tag="pA")
                    for th in range(2):
                        for fh in range(2):
                            nc.tensor.matmul(o2_ps[:, th, :],
                                             lhsT=ot[:, fh, th * 128:(th + 1) * 128],
                                             rhs=wo_t[:, fh, :],
                                             start=fh == 0, stop=fh == 1)
                    for th in range(2):
                        ys_sl = ys_pair[th][:, :].rearrange(
                            "p (g1 w) -> p g1 w", w=128)[:, :, wn * 16:(wn + 1) * 16]
                        nc.vector.tensor_scalar(
                            out=ys_sl,
                            in0=o2_ps[:, th, :].rearrange("p (a b) -> p a b", b=16),
                            scalar1=rec[:, th:th + 1], scalar2=None, op0=OP.mult)

                if use_bo:
                    for ct in range(2):
                        nc.gpsimd.tensor_add(ys_pair[ct], ys_pair[ct].bitcast(F32), bo_st)

                for nb in range(4):
                    chunks = [(q // 8, q % 8) for q in range(nb * 4, nb * 4 + 4)]
                    yt = ffn.tile([128, 2, 512], F32R, tag="yt")
                    for eh in range(2):
                        yt_ps = pA.tile([128, 512], F32, tag="pA")
                        for pos, (ct, j) in enumerate(chunks):
                            nc.tensor.transpose(
                                yt_ps[:, pos * 128:(pos + 1) * 128].bitcast(F32R),
                                ys_pair[ct][:, j * 256 + eh * 128: j * 256 + (eh + 1) * 128],
                                ident)
                        nc.vector.tensor_copy(yt[:, eh, :], yt_ps)

                    hh = ffn.tile([128, 8, 512], F32R, tag="hh")
                    for fp in range(4):
                        h_ps = pH.tile([128, 2, 512], F32, tag="pH")
                        for i in range(2):
                            fm = fp * 2 + i
                            for eh in range(2):
                                nc.tensor.matmul(h_ps[:, i, :],
                                                 lhsT=w1_t[:, eh, fm * 128:(fm + 1) * 128],
                                                 rhs=yt[:, eh, :],
                                                 start=eh == 0, stop=eh == 1)
                        if use_b1:
                            for i in range(2):
                                fm = fp * 2 + i
                                nc.scalar.activation(out=hh[:, fm, :], in_=h_ps[:, i, :],
                                                     func=AF.Gelu,
                                                     bias=b1_t[:, fm:fm + 1])
                        else:
                            nc.scalar.activation(out=hh[:, fp * 2:(fp + 1) * 2, :],
                                                 in_=h_ps, func=AF.Gelu)

                    ft = ffn.tile([128, 2, 512], F32R, tag="ft")
                    for em in range(2):
                        f_ps = pF.tile([128, 512], F32, tag="pF")
                        for fm in range(8):
                            nc.tensor.matmul(f_ps,
                                             lhsT=w2_t[:, fm, em * 128:(em + 1) * 128],
                                             rhs=hh[:, fm, :],
                                             start=fm == 0, stop=fm == 7)
                        if use_b2:
                            nc.scalar.activation(out=ft[:, em, :], in_=f_ps,
                                                 func=AF.Identity,
                                                 bias=b2_t[:, em:em + 1])
                        else:
                            nc.vector.tensor_copy(ft[:, em, :], f_ps)

                    z_ps = []
                    for pp in range(2):
                        zp = pF.tile([128, 2, 256], F32, tag="pF")
                        for i in range(2):
                            pos = pp * 2 + i
                            for em in range(2):
                                nc.tensor.transpose(
                                    zp[:, i, em * 128:(em + 1) * 128].bitcast(F32R),
                                    ft[:, em, pos * 128:(pos + 1) * 128], ident)
                        z_ps.append(zp)

                    mvs1 = msc.tile([128, 4, 2], F32, tag="mvs1")
                    for pos in range(4):
                        bst = msc.tile([128, 6], F32, tag="bst")
                        nc.vector.bn_stats(out=bst, in_=z_ps[pos // 2][:, pos % 2, :])
                        nc.vector.bn_aggr(out=mvs1[:, pos, :], in_=bst)
                    rs1 = newton_rsqrt(mvs1[:, :, 1], 4)

                    y2s = []
                    mvs2 = msc.tile([128, 4, 2], F32, tag="mvs2")
                    for pos, (ct, j) in enumerate(chunks):
                        ln1 = lnp.tile([128, 256], F32, tag="ln1")
                        nc.vector.tensor_scalar(
                            out=ln1, in0=z_ps[pos // 2][:, pos % 2, :],
                            scalar1=mvs1[:, pos, 0:1], scalar2=rs1[:, pos:pos + 1],
                            op0=OP.subtract, op1=OP.mult)
                        if use_g1:
                            nc.gpsimd.tensor_mul(ln1, ln1, g1_bc)
                            nc.gpsimd.tensor_add(ln1, ln1, be1_bc)
                        y2 = lnp.tile([128, 256], F32, tag="y2")
                        nc.gpsimd.tensor_add(
                            y2, ln1,
                            ys_pair[ct][:, j * 256:(j + 1) * 256].bitcast(F32))
                        y2s.append(y2)
                        bst = msc.tile([128, 6], F32, tag="bst")
                        nc.vector.bn_stats(out=bst, in_=y2)
                        nc.vector.bn_aggr(out=mvs2[:, pos, :], in_=bst)
                    rs2 = newton_rsqrt(mvs2[:, :, 1], 4)

                    for pos, (ct, j) in enumerate(chunks):
                        ln2 = lnp.tile([128, 256], F32, tag="ln2")
                        nc.vector.tensor_scalar(
                            out=ln2, in0=y2s[pos],
                            scalar1=mvs2[:, pos, 0:1], scalar2=rs2[:, pos:pos + 1],
                            op0=OP.subtract, op1=OP.mult)
                        if use_g2:
                            nc.gpsimd.tensor_mul(ln2, ln2, g2_bc)
                            nc.gpsimd.tensor_add(ln2, ln2, be2_bc)
                        outt = lnp.tile([128, 256], F32, tag="outt")
                        nc.gpsimd.tensor_add(outt, ln2, y2s[pos])
                        nc.sync.dma_start(
                            out=OUTV[ct * 128:(ct + 1) * 128, hn * 8 + j, :],
                            in_=outt)

    nc.compile()
    return nc


def _inputs_v1(flags, x, Wq, Wk, Wv, Wo, W1, W2, bq, bk, bv, bo, b1, b2,
               g1, be1, g2, be2):
    scale = 1.0 / np.sqrt(np.float32(E))
    base = {
        "wq": _round_f32r(Wq * scale),
        "wk": _round_f32r(Wk),
        "wv": _round_f32r(Wv),
        "wo": _round_f32r(Wo),
        "w1": _round_f32r(W1),
        "w2": _round_f32r(W2),
        "ident": np.eye(128, dtype=np.float32),
    }
    use_bqk, use_bv, use_bo, use_b1, use_b2, use_g1, use_g2 = flags
    if use_bqk:
        base["bq"] = bq * scale
        base["bk"] = bk
    if use_bv:
        base["bv"] = bv
    if use_bo:
        base["bo"] = bo
    if use_b1:
        base["b1"] = b1
    if use_b2:
        base["b2"] = b2
    if use_g1:
        base["g1"] = g1
        base["be1"] = be1
    if use_g2:
        base["g2"] = g2
        base["be2"] = be2
    return [dict(base, x=_round_f32r(x[b])) for b in range(B)]


def _build(flags):
    use_bqk, use_bv, use_bo, use_b1, use_b2, use_g1, use_g2 = flags
    if not (use_bqk or use_bv or use_bo):
        return _build_fast((use_b1, use_b2, use_g1, use_g2)), True
    return _build_v1(flags), False


def _get_program(flags):
    if flags not in _CACHE:
        _CACHE[flags] = _build(flags)
    return _CACHE[flags]


def kernel(**inputs):
    import ml_dtypes

    bf16 = ml_dtypes.bfloat16
    x = np.asarray(inputs["x"], np.float32)
    Wq = np.asarray(inputs["Wq"], np.float32)
    Wk = np.asarray(inputs["Wk"], np.float32)
    Wv = np.asarray(inputs["Wv"], np.float32)
    Wo = np.asarray(inputs["Wo"], np.float32)
    W1 = np.asarray(inputs["W1"], np.float32)
    W2 = np.asarray(inputs["W2"], np.float32)
    bq = np.asarray(inputs["bq"], np.float32)
    bk = np.asarray(inputs["bk"], np.float32)
    bv = np.asarray(inputs["bv"], np.float32)
    bo = np.asarray(inputs["bo"], np.float32)
    b1 = np.asarray(inputs["b1"], np.float32)
    b2 = np.asarray(inputs["b2"], np.float32)
    g1 = np.asarray(inputs["g1"], np.float32)
    be1 = np.asarray(inputs["be1"], np.float32)
    g2 = np.asarray(inputs["g2"], np.float32)
    be2 = np.asarray(inputs["be2"], np.float32)

    flags = (
        bool(bq.any() or bk.any()),
        bool(bv.any()),
        bool(bo.any()),
        bool(b1.any()),
        bool(b2.any()),
        bool((g1 != 1.0).any() or be1.any()),
        bool((g2 != 1.0).any() or be2.any()),
    )
    nc, fast = _get_program(flags)

    scale = 1.0 / np.sqrt(np.float32(E))
    if fast:
        use_bqk, use_bv, use_bo, use_b1, use_b2, use_g1, use_g2 = flags
        M = (Wq @ Wk.T) * scale
        N = Wv @ Wo
        base = {
            "m": M.astype(bf16),
            "n": N.astype(bf16),
            "w1": W1.astype(bf16),
            "w2": W2.astype(bf16),
            "ident": np.eye(128, dtype=np.float32).astype(bf16),
        }
        if use_b1:
            base["b1"] = b1
        if use_b2:
            base["b2"] = b2
        if use_g1:
            base["g1"] = g1
            base["be1"] = be1
        if use_g2:
            base["g2"] = g2
            base["be2"] = be2
        in_maps = [dict(base, x=x[b].astype(bf16)) for b in range(B)]
    else:
        in_maps = _inputs_v1(flags, x, Wq, Wk, Wv, Wo, W1, W2, bq, bk, bv, bo,
                             b1, b2, g1, be1, g2, be2)

    from concourse.bass_utils import run_bass_kernel_spmd

    res = run_bass_kernel_spmd(nc, in_maps, list(range(B)))
    kernel.last_exec_time_ns = res.exec_time_ns
    kernel.last_trace = res.instructions_and_trace
    kernel.last_profile_json = res.profile_json
    return np.stack([r["out"] for r in res.results], axis=0)


# revision 18
# speedup vs baseline: 2.5609x; 2.5609x over previous
"""GridTransformerBlock TRN2 kernel (v2).

Sharding: batch-parallel over B=8 -> one batch per NeuronCore, zero collectives.

Layout: the reference's (B,S,E)->(B,E,H,W) reshape is a raw reinterpret, so per
batch the buffer is 256 channel planes of 128x128; each 16x16 window's
attention tile T is [tokens=channels, features=window pixels].

v2 structure (vs v1):
- bf16 operands everywhere (PSUM accumulation stays fp32); host pre-casts.
- Zero attention biases let us fold weights on the host:
    scores = T (Wq Wk^T / sqrt(E)) T^T   -> one fused M matrix, no K tensor
    y      = A (T (Wv Wo))               -> one fused N matrix, no separate V/O
- Window-pair batching: QKV-side projections run at N=512 moving dim.
- Phase split: attention for all 8 stripes (Exp table only), then FFN for all
  stripes (Gelu table only) -> 2 ACT table loads instead of ~44, and dense
  back-to-back PE streams that keep the HAM clock-gate warm.
- ys (attention output / residual) kept resident in SBUF in bf16 (8 MB).
- Multi-group bn_stats, batched 2-iteration Newton rsqrt, PSUM evacuations
  balanced across Vector and Scalar engines.
"""

import os
import sys
import numpy as np

for _p in ("/opt/trn_rl_repo", "/root/.axon_site/_ro/trn_rl_repo"):
    if _p not in sys.path and os.path.isdir(_p):
        sys.path.insert(0, _p)

B, S, E, FF = 8, 16384, 256, 1024
H, W, G = 128, 128, 16
Hn, Wn = 8, 8

_CACHE = {}


def _build_fast(ffn_flags):
    """Fused-attention build: requires zero attention biases (bq/bk/bv/bo)."""
    use_b1, use_b2, use_g1, use_g2 = ffn_flags
    import concourse.bacc as bacc
    import concourse.mybir as mybir
    import concourse.tile as tile
    from contextlib import ExitStack

    F32 = mybir.dt.float32
    BF16 = mybir.dt.bfloat16
    I32 = mybir.dt.int32
    AF = mybir.ActivationFunctionType
    OP = mybir.AluOpType

    nc = bacc.Bacc("TRN2", target_bir_lowering=False, debug=False, num_devices=8)

    x_d = nc.dram_tensor("x", [S, E], BF16, kind="ExternalInput")
    m_d = nc.dram_tensor("m", [E, E], BF16, kind="ExternalInput")   # Wq Wk^T / 16
    n_d = nc.dram_tensor("n", [E, E], BF16, kind="ExternalInput")   # Wv Wo
    w1_d = nc.dram_tensor("w1", [E, FF], BF16, kind="ExternalInput")
    w2_d = nc.dram_tensor("w2", [FF, E], BF16, kind="ExternalInput")
    id_d = nc.dram_tensor("ident", [128, 128], BF16, kind="ExternalInput")
    out_d = nc.dram_tensor("out", [S, E], F32, kind="ExternalOutput")
    if use_b1:
        b1_d = nc.dram_tensor("b1", [FF], F32, kind="ExternalInput")
    if use_b2:
        b2_d = nc.dram_tensor("b2", [E], F32, kind="ExternalInput")
    if use_g1:
        g1_d = nc.dram_tensor("g1", [E], F32, kind="ExternalInput")
        be1_d = nc.dram_tensor("be1", [E], F32, kind="ExternalInput")
    if use_g2:
        g2_d = nc.dram_tensor("g2", [E], F32, kind="ExternalInput")
        be2_d = nc.dram_tensor("be2", [E], F32, kind="ExternalInput")

    import concourse.bass as bass

    def bcast_ap(dram, n=256):
        return bass.AP(tensor=dram.ap().tensor, offset=0, ap=[[0, 128], [1, n]])

    X = x_d.ap().rearrange("(c t) e -> c (t e)", t=64)      # [256, 16384]
    OUTV = out_d.ap().rearrange("(c t) e -> c t e", t=64)   # [256, 64, 256]

    with tile.TileContext(nc) as tc:
        with ExitStack() as ctx:
            const = ctx.enter_context(tc.tile_pool(name="const", bufs=1))
            xsp = ctx.enter_context(tc.tile_pool(name="xsp", bufs=4))
            ysp = ctx.enter_context(tc.tile_pool(name="ysp", bufs=16))
            twp = ctx.enter_context(tc.tile_pool(name="twp", bufs=2))
            att = ctx.enter_context(tc.tile_pool(name="att", bufs=2))
            stp = ctx.enter_context(tc.tile_pool(name="stp", bufs=2))
            ffn = ctx.enter_context(tc.tile_pool(name="ffn", bufs=2))
            lnp = ctx.enter_context(tc.tile_pool(name="lnp", bufs=3))
            ztp = ctx.enter_context(tc.tile_pool(name="ztp", bufs=6))
            y2p = ctx.enter_context(tc.tile_pool(name="y2p", bufs=6))
            msc = ctx.enter_context(tc.tile_pool(name="msc", bufs=2))
            outp = ctx.enter_context(tc.tile_pool(name="outp", bufs=4))
            p2 = ctx.enter_context(tc.tile_pool(name="p2", bufs=2, space="PSUM"))
            p1 = ctx.enter_context(tc.tile_pool(name="p1", bufs=2, space="PSUM"))
            pb = ctx.enter_context(tc.tile_pool(name="pb", bufs=2, space="PSUM"))

            ident = const.tile([128, 128], BF16)
            nc.sync.dma_start(out=ident, in_=id_d.ap()[:, :])
            m_t = const.tile([128, 2, 256], BF16)
            nc.sync.dma_start(out=m_t, in_=m_d.ap().rearrange("(ph k) g -> k ph g", k=128))
            n_t = const.tile([128, 2, 256], BF16)
            nc.sync.dma_start(out=n_t, in_=n_d.ap().rearrange("(ph k) g -> k ph g", k=128))
            w1_t = const.tile([128, 2, 1024], BF16)
            nc.sync.dma_start(out=w1_t, in_=w1_d.ap().rearrange("(eh k) f -> k eh f", k=128))
            w2_t = const.tile([128, 8, 256], BF16)
            nc.sync.dma_start(out=w2_t, in_=w2_d.ap().rearrange("(fm k) e -> k fm e", k=128))
            if use_b1:
                b1_t = const.tile([128, 8], F32)
                nc.sync.dma_start(out=b1_t, in_=b1_d.ap().rearrange("(fm p) -> p fm", p=128))
            if use_b2:
                b2_t = const.tile([128, 2], F32)
                nc.sync.dma_start(out=b2_t, in_=b2_d.ap().rearrange("(em p) -> p em", p=128))
            if use_g1:
                g1_bc = const.tile([128, 256], F32)
                nc.sync.dma_start(out=g1_bc, in_=bcast_ap(g1_d))
                be1_bc = const.tile([128, 256], F32)
                nc.sync.dma_start(out=be1_bc, in_=bcast_ap(be1_d))
            if use_g2:
                g2_bc = const.tile([128, 256], F32)
                nc.sync.dma_start(out=g2_bc, in_=bcast_ap(g2_d))
                be2_bc = const.tile([128, 256], F32)
                nc.sync.dma_start(out=be2_bc, in_=bcast_ap(be2_d))

            def newton_rsqrt(var_ap, n, tag, iters=2):
                """rstd = 1/sqrt(var + eps) for a [128, n] strided var AP.

                Bitcast magic-constant seed (~3.4% err) + `iters` Newton
                steps on [128, n]; multiplies routed to GpSimd to offload
                the Vector engine.
                """
                w = msc.tile([128, n], F32, tag=f"nw_w{tag}")
                nc.vector.tensor_scalar(out=w, in0=var_ap, scalar1=1e-5,
                                        scalar2=None, op0=OP.add)
                r = msc.tile([128, n], F32, tag=f"nw_r{tag}")
                nc.vector.tensor_scalar(out=r.bitcast(I32), in0=w.bitcast(I32),
                                        scalar1=1, scalar2=None,
                                        op0=OP.logical_shift_right)
                nc.vector.tensor_scalar(out=r.bitcast(I32), in0=r.bitcast(I32),
                                        scalar1=0xFFFFFFFF, scalar2=None,
                                        op0=OP.bitwise_xor)
                nc.vector.tensor_scalar(out=r.bitcast(I32), in0=r.bitcast(I32),
                                        scalar1=0x5F375A86 + 1, scalar2=None,
                                        op0=OP.add)
                rsq = msc.tile([128, n], F32, tag=f"nw_rsq{tag}")
                u = msc.tile([128, n], F32, tag=f"nw_u{tag}")
                v = msc.tile([128, n], F32, tag=f"nw_v{tag}")
                for _ in range(iters):
                    nc.gpsimd.tensor_mul(rsq, r, r)
                    nc.gpsimd.tensor_mul(u, rsq, w)
                    nc.vector.tensor_scalar(out=v, in0=u, scalar1=-0.5, scalar2=1.5,
                                            op0=OP.mult, op1=OP.add)
                    nc.gpsimd.tensor_mul(r, r, v)
                return r

            ys_all = []  # [hn][ct] -> [128, 2048] bf16 tile

            # ================= Phase A: attention, all stripes =================
            for hn in range(Hn):
                xs_pair = []
                for ct in range(2):
                    t = xsp.tile([128, 2048], BF16, tag="xs")
                    nc.sync.dma_start(
                        out=t, in_=X[ct * 128:(ct + 1) * 128, hn * 2048:(hn + 1) * 2048])
                    xs_pair.append(t)
                ys_pair = [ysp.tile([128, 2048], BF16, tag="ys", name=f"ys{hn}_{i}")
                           for i in range(2)]
                ys_all.append(ys_pair)

                for pk in range(4):  # window pair (2pk, 2pk+1)
                    # gather: per-window contiguous [128ch, 256px] blocks
                    t_sb = twp.tile([128, 2, 2, 256], BF16, tag="tw")
                    for ct in range(2):
                        xv = xs_pair[ct][:, :].rearrange("p (g1 w) -> p g1 w", w=128)
                        for wi in range(2):
                            wn = pk * 2 + wi
                            nc.gpsimd.tensor_copy(
                                t_sb[:, ct, wi, :].rearrange(
                                    "p (g1 g2) -> p g1 g2", g2=16),
                                xv[:, :, wn * 16:(wn + 1) * 16])

                    # tt = T^T per window: [px(ph half), wi, ch(2ct*128)]
                    tt_ps = pb.tile([128, 2, 2, 256], BF16, tag="pb")
                    for ct in range(2):
                        for wi in range(2):
                            for ph in range(2):
                                nc.tensor.transpose(
                                    tt_ps[:, ph, wi, ct * 128:(ct + 1) * 128],
                                    t_sb[:, ct, wi, ph * 128:(ph + 1) * 128],
                                    ident)
                    tt = att.tile([128, 2, 2, 256], BF16, tag="tt")
                    nc.vector.tensor_copy(tt, tt_ps)

                    # u^T = M^T T^T : [g(2 chunks gh), (wi, tok)]
                    u_ps = p2.tile([128, 2, 512], F32, tag="p2")
                    for gh in range(2):
                        for ph in range(2):
                            nc.tensor.matmul(u_ps[:, gh, :],
                                             lhsT=m_t[:, ph, gh * 128:(gh + 1) * 128],
                                             rhs=tt[:, ph, :, :],
                                             start=ph == 0, stop=ph == 1)
                    ut = att.tile([128, 2, 512], BF16, tag="ut")
                    nc.vector.tensor_copy(ut, u_ps)

                    # u2 = T N : [(wi, tok-chunk ct), fo]
                    u2_ps = p2.tile([128, 2, 2, 256], F32, tag="p2")
                    for wi in range(2):
                        for ct in range(2):
                            for ph in range(2):
                                nc.tensor.matmul(
                                    u2_ps[:, wi, ct, :],
                                    lhsT=tt[:, ph, wi, ct * 128:(ct + 1) * 128],
                                    rhs=n_t[:, ph, :],
                                    start=ph == 0, stop=ph == 1)
                    u2 = att.tile([128, 2, 2, 256], BF16, tag="u2")
                    nc.scalar.activation(out=u2, in_=u2_ps, func=AF.Copy)

                    # scores + exp (unnormalized), denominator accumulated
                    den = stp.tile([128, 4], F32, tag="den")
                    aa = att.tile([128, 2, 2, 256], BF16, tag="aa")
                    for wi in range(2):
                        s_ps = p1.tile([128, 2, 256], F32, tag="p1")
                        for th in range(2):
                            for gh in range(2):
                                nc.tensor.matmul(
                                    s_ps[:, th, :],
                                    lhsT=ut[:, gh, wi * 256 + th * 128:
                                            wi * 256 + (th + 1) * 128],
                                    rhs=tt[:, gh, wi, :],
                                    start=gh == 0, stop=gh == 1)
                        for th in range(2):
                            nc.scalar.activation(
                                out=aa[:, wi, th, :], in_=s_ps[:, th, :],
                                func=AF.Exp,
                                accum_out=den[:, wi * 2 + th:wi * 2 + th + 1])
                    rec = stp.tile([128, 4], F32, tag="rec")
                    nc.vector.reciprocal(rec, den)

                    # at = A^T (unnormalized): [(wi), k-chunk t2h, q]
                    at_ps = pb.tile([128, 2, 2, 256], BF16, tag="pb")
                    for wi in range(2):
                        for th in range(2):
                            for t2h in range(2):
                                nc.tensor.transpose(
                                    at_ps[:, wi, t2h, th * 128:(th + 1) * 128],
                                    aa[:, wi, th, t2h * 128:(t2h + 1) * 128],
                                    ident)
                    at = att.tile([128, 2, 2, 256], BF16, tag="at")
                    nc.vector.tensor_copy(at, at_ps)

                    # y_window = A u2, scaled by 1/den at evacuation
                    for wi in range(2):
                        o_ps = p1.tile([128, 2, 256], F32, tag="p1")
                        for th in range(2):
                            for t2h in range(2):
                                nc.tensor.matmul(
                                    o_ps[:, th, :],
                                    lhsT=at[:, wi, t2h, th * 128:(th + 1) * 128],
                                    rhs=u2[:, wi, t2h, :],
                                    start=t2h == 0, stop=t2h == 1)
                        wn = pk * 2 + wi
                        for th in range(2):
                            ys_sl = ys_pair[th][:, :].rearrange(
                                "p (g1 w) -> p g1 w", w=128)[:, :, wn * 16:(wn + 1) * 16]
                            if th == 0:
                                nc.vector.tensor_scalar(
                                    out=ys_sl,
                                    in0=o_ps[:, th, :].rearrange(
                                        "p (a b) -> p a b", b=16),
                                    scalar1=rec[:, wi * 2 + th:wi * 2 + th + 1],
                                    scalar2=None, op0=OP.mult)
                            else:
                                nc.scalar.activation(
                                    out=ys_sl,
                                    in_=o_ps[:, th, :].rearrange(
                                        "p (a b) -> p a b", b=16),
                                    func=AF.Copy,
                                    scale=rec[:, wi * 2 + th:wi * 2 + th + 1])

            # ================= Phase B: FFN + LNs, all stripes =================
            for hn in range(Hn):
                ys_pair = ys_all[hn]
                z_tiles = []
                mvs1 = msc.tile([128, 16, 2], F32, tag="mvs1")
                for nb in range(4):
                    chunks = [(q // 8, q % 8) for q in range(nb * 4, nb * 4 + 4)]
                    yt = ffn.tile([128, 2, 512], BF16, tag="yt")
                    yt_ps = pb.tile([128, 2, 512], BF16, tag="pb")
                    for eh in range(2):
                        for pos, (ct, j) in enumerate(chunks):
                            nc.tensor.transpose(
                                yt_ps[:, eh, pos * 128:(pos + 1) * 128],
                                ys_pair[ct][:, j * 256 + eh * 128:
                                            j * 256 + (eh + 1) * 128],
                                ident)
                    nc.vector.tensor_copy(yt, yt_ps)

                    hh = ffn.tile([128, 8, 512], BF16, tag="hh")
                    for fp in range(4):
                        h_ps = p2.tile([128, 2, 512], F32, tag="p2")
                        for i in range(2):
                            fm = fp * 2 + i
                            for eh in range(2):
                                nc.tensor.matmul(h_ps[:, i, :],
                                                 lhsT=w1_t[:, eh, fm * 128:(fm + 1) * 128],
                                                 rhs=yt[:, eh, :],
                                                 start=eh == 0, stop=eh == 1)
                        if use_b1:
                            for i in range(2):
                                fm = fp * 2 + i
                                nc.scalar.activation(out=hh[:, fm, :], in_=h_ps[:, i, :],
                                                     func=AF.Gelu,
                                                     bias=b1_t[:, fm:fm + 1])
                        else:
                            nc.scalar.activation(out=hh[:, fp * 2:(fp + 1) * 2, :],
                                                 in_=h_ps, func=AF.Gelu)

                    ft = ffn.tile([128, 2, 512], BF16, tag="ft")
                    for em in range(2):
                        f_ps = p1.tile([128, 512], F32, tag="p1")
                        for fm in range(8):
                            nc.tensor.matmul(f_ps,
                                             lhsT=w2_t[:, fm, em * 128:(em + 1) * 128],
                                             rhs=hh[:, fm, :],
                                             start=fm == 0, stop=fm == 7)
                        if use_b2:
                            nc.scalar.activation(out=ft[:, em, :], in_=f_ps,
                                                 func=AF.Identity,
                                                 bias=b2_t[:, em:em + 1])
                        else:
                            nc.scalar.activation(out=ft[:, em, :], in_=f_ps,
                                                 func=AF.Copy)

                    z_ps = pb.tile([128, 4, 256], BF16, tag="pb",
                                   name=f"z{hn}_{nb}")
                    for pos in range(4):
                        for em in range(2):
                            nc.tensor.transpose(
                                z_ps[:, pos, em * 128:(em + 1) * 128],
                                ft[:, em, pos * 128:(pos + 1) * 128],
                                ident)
                    zt = ztp.tile([128, 4, 256], BF16, tag="zt",
                                  name=f"zt{hn}_{nb}")
                    nc.vector.tensor_copy(zt, z_ps)
                    z_tiles.append(zt)

                    bst1 = msc.tile([128, 6], F32, tag="bst1")
                    for pos in range(4):
                        nc.vector.bn_stats(out=bst1, in_=zt[:, pos, :])
                        nc.vector.bn_aggr(out=mvs1[:, nb * 4 + pos, :], in_=bst1)

                rs1 = newton_rsqrt(mvs1[:, :, 1], 16, "a", iters=2)
                # per-partition bias for the ACT-side LN1 apply: -m1*rs1
                mb1 = msc.tile([128, 16], F32, tag="mb1")
                nc.gpsimd.tensor_mul(mb1, mvs1[:, :, 0], rs1)
                nc.vector.tensor_scalar(out=mb1, in0=mb1, scalar1=-1.0,
                                        scalar2=None, op0=OP.mult)

                mvs2 = msc.tile([128, 16, 2], F32, tag="mvs2")
                y2_tiles = []
                for nb in range(4):
                    chunks = [(q // 8, q % 8) for q in range(nb * 4, nb * 4 + 4)]
                    zt = z_tiles[nb]
                    y2 = y2p.tile([128, 4, 256], BF16, tag="y2",
                                  name=f"y2{hn}_{nb}")
                    bst2 = msc.tile([128, 6], F32, tag="bst2")
                    for pos, (ct, j) in enumerate(chunks):
                        k = nb * 4 + pos
                        ln1 = lnp.tile([128, 256], BF16, tag="ln1")
                        nc.scalar.activation(
                            out=ln1, in_=zt[:, pos, :], func=AF.Identity,
                            scale=rs1[:, k:k + 1], bias=mb1[:, k:k + 1])
                        if use_g1:
                            nc.gpsimd.tensor_mul(ln1, ln1, g1_bc)
                            nc.gpsimd.tensor_add(ln1, ln1, be1_bc)
                        nc.gpsimd.tensor_add(
                            y2[:, pos, :], ln1,
                            ys_pair[ct][:, j * 256:(j + 1) * 256])
                        nc.vector.bn_stats(out=bst2, in_=y2[:, pos, :])
                        nc.vector.bn_aggr(out=mvs2[:, k, :], in_=bst2)
                    y2_tiles.append(y2)

                rs2 = newton_rsqrt(mvs2[:, :, 1], 16, "b", iters=2)
                # out = y2 + ln2 = y2*(1+rs2) - m2*rs2  (one fused op per chunk)
                a2 = msc.tile([128, 16], F32, tag="a2")
                nc.vector.tensor_scalar(out=a2, in0=rs2, scalar1=1.0,
                                        scalar2=None, op0=OP.add)
                b2m = msc.tile([128, 16], F32, tag="b2m")
                nc.gpsimd.tensor_mul(b2m, mvs2[:, :, 0], rs2)

                for nb in range(4):
                    chunks = [(q // 8, q % 8) for q in range(nb * 4, nb * 4 + 4)]
                    y2 = y2_tiles[nb]
                    for pos, (ct, j) in enumerate(chunks):
                        k = nb * 4 + pos
                        if use_g2:
                            ln2 = lnp.tile([128, 256], BF16, tag="ln2")
                            nc.vector.tensor_scalar(
                                out=ln2, in0=y2[:, pos, :],
                                scalar1=mvs2[:, k, 0:1], scalar2=rs2[:, k:k + 1],
                                op0=OP.subtract, op1=OP.mult)
                            nc.gpsimd.tensor_mul(ln2, ln2, g2_bc)
                            nc.gpsimd.tensor_add(ln2, ln2, be2_bc)
                            outt = outp.tile([128, 256], F32, tag="outt")
                            nc.gpsimd.tensor_add(outt, ln2, y2[:, pos, :])
                        else:
                            outt = outp.tile([128, 256], F32, tag="outt")
                            nc.vector.tensor_scalar(
                                out=outt, in0=y2[:, pos, :],
                                scalar1=a2[:, k:k + 1], scalar2=b2m[:, k:k + 1],
                                op0=OP.mult, op1=OP.subtract)
                        nc.scalar.dma_start(
                            out=OUTV[ct * 128:(ct + 1) * 128, hn * 8 + j, :],
                            in_=outt)

    nc.compile()
    return nc


def _round_f32r(x):
    u = np.ascontiguousarray(x, np.float32).view(np.uint32)
    return ((u + np.uint32(0x800)) & np.uint32(0xFFFFF000)).view(np.float32)


def _build_v1(flags):
    """Unfused fallback (handles attention biases); f32r, per-window."""
    use_bqk, use_bv, use_bo, use_b1, use_b2, use_g1, use_g2 = flags
    import concourse.bacc as bacc
    import concourse.mybir as mybir
    import concourse.tile as tile
    from contextlib import ExitStack

    F32 = mybir.dt.float32
    F32R = mybir.dt.float32r
    I32 = mybir.dt.int32
    AF = mybir.ActivationFunctionType
    OP = mybir.AluOpType

    nc = bacc.Bacc("TRN2", target_bir_lowering=False, debug=False, num_devices=8)

    x_d = nc.dram_tensor("x", [S, E], F32R, kind="ExternalInput")
    wq_d = nc.dram_tensor("wq", [E, E], F32R, kind="ExternalInput")
    wk_d = nc.dram_tensor("wk", [E, E], F32R, kind="ExternalInput")
    wv_d = nc.dram_tensor("wv", [E, E], F32R, kind="ExternalInput")
    wo_d = nc.dram_tensor("wo", [E, E], F32R, kind="ExternalInput")
    w1_d = nc.dram_tensor("w1", [E, FF], F32R, kind="ExternalInput")
    w2_d = nc.dram_tensor("w2", [FF, E], F32R, kind="ExternalInput")
    id_d = nc.dram_tensor("ident", [128, 128], F32R, kind="ExternalInput")
    out_d = nc.dram_tensor("out", [S, E], F32, kind="ExternalOutput")
    if use_bqk:
        bq_d = nc.dram_tensor("bq", [E], F32, kind="ExternalInput")
        bk_d = nc.dram_tensor("bk", [E], F32, kind="ExternalInput")
    if use_bv:
        bv_d = nc.dram_tensor("bv", [E], F32, kind="ExternalInput")
    if use_bo:
        bo_d = nc.dram_tensor("bo", [E], F32, kind="ExternalInput")
    if use_b1:
        b1_d = nc.dram_tensor("b1", [FF], F32, kind="ExternalInput")
    if use_b2:
        b2_d = nc.dram_tensor("b2", [E], F32, kind="ExternalInput")
    if use_g1:
        g1_d = nc.dram_tensor("g1", [E], F32, kind="ExternalInput")
        be1_d = nc.dram_tensor("be1", [E], F32, kind="ExternalInput")
    if use_g2:
        g2_d = nc.dram_tensor("g2", [E], F32, kind="ExternalInput")
        be2_d = nc.dram_tensor("be2", [E], F32, kind="ExternalInput")

    import concourse.bass as bass

    def bcast_ap(dram, n=256):
        return bass.AP(tensor=dram.ap().tensor, offset=0, ap=[[0, 128], [1, n]])

    X = x_d.ap().rearrange("(c t) e -> c (t e)", t=64)
    OUTV = out_d.ap().rearrange("(c t) e -> c t e", t=64)

    with tile.TileContext(nc) as tc:
        with ExitStack() as ctx:
            const = ctx.enter_context(tc.tile_pool(name="const", bufs=1))
            xsp = ctx.enter_context(tc.tile_pool(name="xsp", bufs=4))
            ysp = ctx.enter_context(tc.tile_pool(name="ysp", bufs=4))
            twp = ctx.enter_context(tc.tile_pool(name="twp", bufs=2))
            att = ctx.enter_context(tc.tile_pool(name="att", bufs=2))
            stp = ctx.enter_context(tc.tile_pool(name="stp", bufs=4))
            ffn = ctx.enter_context(tc.tile_pool(name="ffn", bufs=2))
            lnp = ctx.enter_context(tc.tile_pool(name="lnp", bufs=4))
            msc = ctx.enter_context(tc.tile_pool(name="msc", bufs=4))
            pA = ctx.enter_context(tc.tile_pool(name="pA", bufs=3, space="PSUM"))
            pH = ctx.enter_context(tc.tile_pool(name="pH", bufs=1, space="PSUM"))
            pF = ctx.enter_context(tc.tile_pool(name="pF", bufs=3, space="PSUM"))

            ident = const.tile([128, 128], F32R)
            nc.sync.dma_start(out=ident, in_=id_d.ap()[:, :])
            wq_t = const.tile([128, 2, 256], F32R)
            wk_t = const.tile([128, 2, 256], F32R)
            wv_t = const.tile([128, 2, 256], F32R)
            wo_t = const.tile([128, 2, 256], F32R)
            for t, d in ((wq_t, wq_d), (wk_t, wk_d), (wv_t, wv_d), (wo_t, wo_d)):
                nc.sync.dma_start(out=t, in_=d.ap().rearrange("(eh k) f -> k eh f", k=128))
            w1_t = const.tile([128, 2, 1024], F32R)
            nc.sync.dma_start(out=w1_t, in_=w1_d.ap().rearrange("(eh k) f -> k eh f", k=128))
            w2_t = const.tile([128, 8, 256], F32R)
            nc.sync.dma_start(out=w2_t, in_=w2_d.ap().rearrange("(fm k) e -> k fm e", k=128))
            if use_bqk:
                bq_t = const.tile([128, 2], F32)
                nc.sync.dma_start(out=bq_t, in_=bq_d.ap().rearrange("(fh p) -> p fh", p=128))
                bk_t = const.tile([128, 2], F32)
                nc.sync.dma_start(out=bk_t, in_=bk_d.ap().rearrange("(fh p) -> p fh", p=128))
            if use_bv:
                bv_bc = const.tile([128, 2, 256], F32)
                nc.sync.dma_start(
                    out=bv_bc,
                    in_=bass.AP(tensor=bv_d.ap().tensor, offset=0,
                                ap=[[0, 128], [0, 2], [1, 256]]))
            if use_bo:
                bo_st = const.tile([128, 2048], F32)
                nc.sync.dma_start(
                    out=bo_st.rearrange("p (g1 wn g2) -> p g1 wn g2", wn=8, g2=16),
                    in_=bass.AP(tensor=bo_d.ap().tensor, offset=0,
                                ap=[[0, 128], [16, 16], [0, 8], [1, 16]]))
            if use_b1:
                b1_t = const.tile([128, 8], F32)
                nc.sync.dma_start(out=b1_t, in_=b1_d.ap().rearrange("(fm p) -> p fm", p=128))
            if use_b2:
                b2_t = const.tile([128, 2], F32)
                nc.sync.dma_start(out=b2_t, in_=b2_d.ap().rearrange("(em p) -> p em", p=128))
            if use_g1:
                g1_bc = const.tile([128, 256], F32)
                nc.sync.dma_start(out=g1_bc, in_=bcast_ap(g1_d))
                be1_bc = const.tile([128, 256], F32)
                nc.sync.dma_start(out=be1_bc, in_=bcast_ap(be1_d))
            if use_g2:
                g2_bc = const.tile([128, 256], F32)
                nc.sync.dma_start(out=g2_bc, in_=bcast_ap(g2_d))
                be2_bc = const.tile([128, 256], F32)
                nc.sync.dma_start(out=be2_bc, in_=bcast_ap(be2_d))

            def newton_rsqrt(var_ap, n):
                w = msc.tile([128, n], F32, tag="nw_w")
                nc.vector.tensor_scalar(out=w, in0=var_ap, scalar1=1e-5,
                                        scalar2=None, op0=OP.add)
                r = msc.tile([128, n], F32, tag="nw_r")
                nc.vector.tensor_scalar(out=r.bitcast(I32), in0=w.bitcast(I32),
                                        scalar1=1, scalar2=None,
                                        op0=OP.logical_shift_right)
                nc.vector.tensor_scalar(out=r.bitcast(I32), in0=r.bitcast(I32),
                                        scalar1=0xFFFFFFFF, scalar2=None,
                                        op0=OP.bitwise_xor)
                nc.vector.tensor_scalar(out=r.bitcast(I32), in0=r.bitcast(I32),
                                        scalar1=0x5F375A86 + 1, scalar2=None,
                                        op0=OP.add)
                rsq = msc.tile([128, n], F32, tag="nw_rsq")
                u = msc.tile([128, n], F32, tag="nw_u")
                v = msc.tile([128, n], F32, tag="nw_v")
                for _ in range(3):
                    nc.vector.tensor_mul(rsq, r, r)
                    nc.vector.tensor_mul(u, rsq, w)
                    nc.vector.tensor_scalar(out=v, in0=u, scalar1=-0.5, scalar2=1.5,
                                            op0=OP.mult, op1=OP.add)
                    nc.vector.tensor_mul(r, r, v)
                return r

            for hn in range(Hn):
                xs_pair = []
                for ct in range(2):
                    t = xsp.tile([128, 2048], F32R, tag="xs")
                    nc.sync.dma_start(
                        out=t, in_=X[ct * 128:(ct + 1) * 128, hn * 2048:(hn + 1) * 2048])
                    xs_pair.append(t)
                ys_pair = [ysp.tile([128, 2048], F32R, tag="ys", name=f"ys{hn}_{i}")
                           for i in range(2)]

                for wn in range(Wn):
                    t_sb = twp.tile([128, 2, 256], F32R, tag="tw")
                    for ct in range(2):
                        xv = xs_pair[ct][:, :].rearrange("p (g1 w) -> p g1 w", w=128)
                        nc.gpsimd.tensor_copy(
                            t_sb[:, ct, :].rearrange("p (g1 g2) -> p g1 g2", g2=16),
                            xv[:, :, wn * 16:(wn + 1) * 16])
                    tt_ps = pA.tile([128, 2, 256], F32, tag="pA")
                    for eh in range(2):
                        for ct in range(2):
                            nc.tensor.transpose(
                                tt_ps[:, eh, ct * 128:(ct + 1) * 128].bitcast(F32R),
                                t_sb[:, ct, eh * 128:(eh + 1) * 128], ident)
                    tt = att.tile([128, 2, 256], F32R, tag="tt")
                    nc.vector.tensor_copy(tt, tt_ps)

                    qt_ps = pA.tile([128, 2, 256], F32, tag="pA")
                    for fh in range(2):
                        for eh in range(2):
                            nc.tensor.matmul(qt_ps[:, fh, :],
                                             lhsT=wq_t[:, eh, fh * 128:(fh + 1) * 128],
                                             rhs=tt[:, eh, :],
                                             start=eh == 0, stop=eh == 1)
                    qt = att.tile([128, 2, 256], F32R, tag="qt")
                    if use_bqk:
                        for fh in range(2):
                            nc.scalar.activation(out=qt[:, fh, :], in_=qt_ps[:, fh, :],
                                                 func=AF.Identity,
                                                 bias=bq_t[:, fh:fh + 1])
                    else:
                        nc.vector.tensor_copy(qt, qt_ps)

                    kt_ps = pA.tile([128, 2, 256], F32, tag="pA")
                    for fh in range(2):
                        for eh in range(2):
                            nc.tensor.matmul(kt_ps[:, fh, :],
                                             lhsT=wk_t[:, eh, fh * 128:(fh + 1) * 128],
                                             rhs=tt[:, eh, :],
                                             start=eh == 0, stop=eh == 1)
                    kt = att.tile([128, 2, 256], F32R, tag="kt")
                    if use_bqk:
                        for fh in range(2):
                            nc.scalar.activation(out=kt[:, fh, :], in_=kt_ps[:, fh, :],
                                                 func=AF.Identity,
                                                 bias=bk_t[:, fh:fh + 1])
                    else:
                        nc.vector.tensor_copy(kt, kt_ps)

                    v_ps = pA.tile([128, 2, 256], F32, tag="pA")
                    for ch in range(2):
                        for eh in range(2):
                            nc.tensor.matmul(v_ps[:, ch, :],
                                             lhsT=tt[:, eh, ch * 128:(ch + 1) * 128],
                                             rhs=wv_t[:, eh, :],
                                             start=eh == 0, stop=eh == 1)
                    vv = att.tile([128, 2, 256], F32R, tag="vv")
                    if use_bv:
                        nc.vector.tensor_add(vv, v_ps, bv_bc)
                    else:
                        nc.scalar.activation(out=vv, in_=v_ps, func=AF.Copy)

                    s_ps = pA.tile([128, 2, 256], F32, tag="pA")
                    for th in range(2):
                        for fh in range(2):
                            nc.tensor.matmul(s_ps[:, th, :],
                                             lhsT=qt[:, fh, th * 128:(th + 1) * 128],
                                             rhs=kt[:, fh, :],
                                             start=fh == 0, stop=fh == 1)
                    aa = att.tile([128, 2, 256], F32R, tag="aa")
                    den = stp.tile([128, 2], F32, tag="den")
                    for th in range(2):
                        nc.scalar.activation(out=aa[:, th, :], in_=s_ps[:, th, :],
                                             func=AF.Exp,
                                             accum_out=den[:, th:th + 1])
                    rec = stp.tile([128, 2], F32, tag="rec")
                    nc.vector.reciprocal(rec, den)

                    at_ps = pA.tile([128, 2, 256], F32, tag="pA")
                    for t2h in range(2):
                        for th in range(2):
                            nc.tensor.transpose(
                                at_ps[:, t2h, th * 128:(th + 1) * 128].bitcast(F32R),
                                aa[:, th, t2h * 128:(t2h + 1) * 128], ident)
                    at = att.tile([128, 2, 256], F32R, tag="at")
                    nc.scalar.activation(out=at, in_=at_ps, func=AF.Copy)

                    ot_ps = pA.tile([128, 2, 256], F32, tag="pA")
                    for fh in range(2):
                        for t2h in range(2):
                            nc.tensor.matmul(ot_ps[:, fh, :],
                                             lhsT=vv[:, t2h, fh * 128:(fh + 1) * 128],
                                             rhs=at[:, t2h, :],
                                             start=t2h == 0, stop=t2h == 1)
                    ot = att.tile([128, 2, 256], F32R, tag="ot")
                    nc.scalar.activation(out=ot, in_=ot_ps, func=AF.Copy)

                    o2_ps = pA.tile([128, 2, 256], F32, tag="pA")
                    for th in range(2):
                        for fh in range(2):
                            nc.tensor.matmul(o2_ps[:, th, :],
                                             lhsT=ot[:, fh, th * 128:(th + 1) * 128],
                                             rhs=wo_t[:, fh, :],
                                             start=fh == 0, stop=fh == 1)
                    for th in range(2):
                        ys_sl = ys_pair[th][:, :].rearrange(
                            "p (g1 w) -> p g1 w", w=128)[:, :, wn * 16:(wn + 1) * 16]
                        nc.vector.tensor_scalar(
                            out=ys_sl,
                            in0=o2_ps[:, th, :].rearrange("p (a b) -> p a b", b=16),
                            scalar1=rec[:, th:th + 1], scalar2=None, op0=OP.mult)

                if use_bo:
                    for ct in range(2):
                        nc.gpsimd.tensor_add(ys_pair[ct], ys_pair[ct].bitcast(F32), bo_st)

                for nb in range(4):
                    chunks = [(q // 8, q % 8) for q in range(nb * 4, nb * 4 + 4)]
                    yt = ffn.tile([128, 2, 512], F32R, tag="yt")
                    for eh in range(2):
                        yt_ps = pA.tile([128, 512], F32, tag="pA")
                        for pos, (ct, j) in enumerate(chunks):
                            nc.tensor.transpose(
                                yt_ps[:, pos * 128:(pos + 1) * 128].bitcast(F32R),
                                ys_pair[ct][:, j * 256 + eh * 128: j * 256 + (eh + 1) * 128],
                                ident)
                        nc.vector.tensor_copy(yt[:, eh, :], yt_ps)

                    hh = ffn.tile([128, 8, 512], F32R, tag="hh")
                    for fp in range(4):
                        h_ps = pH.tile([128, 2, 512], F32, tag="pH")
                        for i in range(2):
                            fm = fp * 2 + i
                            for eh in range(2):
                                nc.tensor.matmul(h_ps[:, i, :],
                                                 lhsT=w1_t[:, eh, fm * 128:(fm + 1) * 128],
                                                 rhs=yt[:, eh, :],
                                                 start=eh == 0, stop=eh == 1)
                        if use_b1:
                            for i in range(2):
                                fm = fp * 2 + i
                                nc.scalar.activation(out=hh[:, fm, :], in_=h_ps[:, i, :],
                                                     func=AF.Gelu,
                                                     bias=b1_t[:, fm:fm + 1])
                        else:
                            nc.scalar.activation(out=hh[:, fp * 2:(fp + 1) * 2, :],
                                                 in_=h_ps, func=AF.Gelu)

                    ft = ffn.tile([128, 2, 512], F32R, tag="ft")
                    for em in range(2):
                        f_ps = pF.tile([128, 512], F32, tag="pF")
                        for fm in range(8):
                            nc.tensor.matmul(f_ps,
                                             lhsT=w2_t[:, fm, em * 128:(em + 1) * 128],
                                             rhs=hh[:, fm, :],
                                             start=fm == 0, stop=fm == 7)
                        if use_b2:
                            nc.scalar.activation(out=ft[:, em, :], in_=f_ps,
                                                 func=AF.Identity,
                                                 bias=b2_t[:, em:em + 1])
                        else:
                            nc.vector.tensor_copy(ft[:, em, :], f_ps)

                    z_ps = []
                    for pp in range(2):
                        zp = pF.tile([128, 2, 256], F32, tag="pF")
                        for i in range(2):
                            pos = pp * 2 + i
                            for em in range(2):
                                nc.tensor.transpose(
                                    zp[:, i, em * 128:(em + 1) * 128].bitcast(F32R),
                                    ft[:, em, pos * 128:(pos + 1) * 128], ident)
                        z_ps.append(zp)

                    mvs1 = msc.tile([128, 4, 2], F32, tag="mvs1")
                    for pos in range(4):
                        bst = msc.tile([128, 6], F32, tag="bst")
                        nc.vector.bn_stats(out=bst, in_=z_ps[pos // 2][:, pos % 2, :])
                        nc.vector.bn_aggr(out=mvs1[:, pos, :], in_=bst)
                    rs1 = newton_rsqrt(mvs1[:, :, 1], 4)

                    y2s = []
                    mvs2 = msc.tile([128, 4, 2], F32, tag="mvs2")
                    for pos, (ct, j) in enumerate(chunks):
                        ln1 = lnp.tile([128, 256], F32, tag="ln1")
                        nc.vector.tensor_scalar(
                            out=ln1, in0=z_ps[pos // 2][:, pos % 2, :],
                            scalar1=mvs1[:, pos, 0:1], scalar2=rs1[:, pos:pos + 1],
                            op0=OP.subtract, op1=OP.mult)
                        if use_g1:
                            nc.gpsimd.tensor_mul(ln1, ln1, g1_bc)
                            nc.gpsimd.tensor_add(ln1, ln1, be1_bc)
                        y2 = lnp.tile([128, 256], F32, tag="y2")
                        nc.gpsimd.tensor_add(
                            y2, ln1,
                            ys_pair[ct][:, j * 256:(j + 1) * 256].bitcast(F32))
                        y2s.append(y2)
                        bst = msc.tile([128, 6], F32, tag="bst")
                        nc.vector.bn_stats(out=bst, in_=y2)
                        nc.vector.bn_aggr(out=mvs2[:, pos, :], in_=bst)
                    rs2 = newton_rsqrt(mvs2[:, :, 1], 4)

                    for pos, (ct, j) in enumerate(chunks):
                        ln2 = lnp.tile([128, 256], F32, tag="ln2")
                        nc.vector.tensor_scalar(
                            out=ln2, in0=y2s[pos],
                            scalar1=mvs2[:, pos, 0:1], scalar2=rs2[:, pos:pos + 1],
                            op0=OP.subtract, op1=OP.mult)
                        if use_g2:
                            nc.gpsimd.tensor_mul(ln2, ln2, g2_bc)
                            nc.gpsimd.tensor_add(ln2, ln2, be2_bc)
                        outt = lnp.tile([128, 256], F32, tag="outt")
                        nc.gpsimd.tensor_add(outt, ln2, y2s[pos])
                        nc.sync.dma_start(
                            out=OUTV[ct * 128:(ct + 1) * 128, hn * 8 + j, :],
                            in_=outt)

    nc.compile()
    return nc


def _inputs_v1(flags, x, Wq, Wk, Wv, Wo, W1, W2, bq, bk, bv, bo, b1, b2,
               g1, be1, g2, be2):
    scale = 1.0 / np.sqrt(np.float32(E))
    base = {
        "wq": _round_f32r(Wq * scale),
        "wk": _round_f32r(Wk),
        "wv": _round_f32r(Wv),
        "wo": _round_f32r(Wo),
        "w1": _round_f32r(W1),
        "w2": _round_f32r(W2),
        "ident": np.eye(128, dtype=np.float32),
    }
    use_bqk, use_bv, use_bo, use_b1, use_b2, use_g1, use_g2 = flags
    if use_bqk:
        base["bq"] = bq * scale
        base["bk"] = bk
    if use_bv:
        base["bv"] = bv
    if use_bo:
        base["bo"] = bo
    if use_b1:
        base["b1"] = b1
    if use_b2:
        base["b2"] = b2
    if use_g1:
        base["g1"] = g1
        base["be1"] = be1
    if use_g2:
        base["g2"] = g2
        base["be2"] = be2
    return [dict(base, x=_round_f32r(x[b])) for b in range(B)]


def _build(flags):
    use_bqk, use_bv, use_bo, use_b1, use_b2, use_g1, use_g2 = flags
    if not (use_bqk or use_bv or use_bo):
        return _build_fast((use_b1, use_b2, use_g1, use_g2)), True
    return _build_v1(flags), False


def _get_program(flags):
    if flags not in _CACHE:
        _CACHE[flags] = _build(flags)
    return _CACHE[flags]


def kernel(**inputs):
    import ml_dtypes

    bf16 = ml_dtypes.bfloat16
    x = np.asarray(inputs["x"], np.float32)
    Wq = np.asarray(inputs["Wq"], np.float32)
    Wk = np.asarray(inputs["Wk"], np.float32)
    Wv = np.asarray(inputs["Wv"], np.float32)
    Wo = np.asarray(inputs["Wo"], np.float32)
    W1 = np.asarray(inputs["W1"], np.float32)
    W2 = np.asarray(inputs["W2"], np.float32)
    bq = np.asarray(inputs["bq"], np.float32)
    bk = np.asarray(inputs["bk"], np.float32)
    bv = np.asarray(inputs["bv"], np.float32)
    bo = np.asarray(inputs["bo"], np.float32)
    b1 = np.asarray(inputs["b1"], np.float32)
    b2 = np.asarray(inputs["b2"], np.float32)
    g1 = np.asarray(inputs["g1"], np.float32)
    be1 = np.asarray(inputs["be1"], np.float32)
    g2 = np.asarray(inputs["g2"], np.float32)
    be2 = np.asarray(inputs["be2"], np.float32)

    flags = (
        bool(bq.any() or bk.any()),
        bool(bv.any()),
        bool(bo.any()),
        bool(b1.any()),
        bool(b2.any()),
        bool((g1 != 1.0).any() or be1.any()),
        bool((g2 != 1.0).any() or be2.any()),
    )
    nc, fast = _get_program(flags)

    scale = 1.0 / np.sqrt(np.float32(E))
    if fast:
        use_bqk, use_bv, use_bo, use_b1, use_b2, use_g1, use_g2 = flags
        M = (Wq @ Wk.T) * scale
        N = Wv @ Wo
        base = {
            "m": M.astype(bf16),
            "n": N.astype(bf16),
            "w1": W1.astype(bf16),
            "w2": W2.astype(bf16),
            "ident": np.eye(128, dtype=np.float32).astype(bf16),
        }
        if use_b1:
            base["b1"] = b1
        if use_b2:
            base["b2"] = b2
        if use_g1:
            base["g1"] = g1
            base["be1"] = be1
        if use_g2:
            base["g2"] = g2
            base["be2"] = be2
        in_maps = [dict(base, x=x[b].astype(bf16)) for b in range(B)]
    else:
        in_maps = _inputs_v1(flags, x, Wq, Wk, Wv, Wo, W1, W2, bq, bk, bv, bo,
                             b1, b2, g1, be1, g2, be2)

    from concourse.bass_utils import run_bass_kernel_spmd

    res = run_bass_kernel_spmd(nc, in_maps, list(range(B)))
    kernel.last_exec_time_ns = res.exec_time_ns
    kernel.last_trace = res.instructions_and_trace
    kernel.last_profile_json = res.profile_json
    return np.stack([r["out"] for r in res.results], axis=0)


# revision 19
# speedup vs baseline: 2.9772x; 1.1625x over previous
"""GridTransformerBlock TRN2 kernel (v2).

Sharding: batch-parallel over B=8 -> one batch per NeuronCore, zero collectives.

Layout: the reference's (B,S,E)->(B,E,H,W) reshape is a raw reinterpret, so per
batch the buffer is 256 channel planes of 128x128; each 16x16 window's
attention tile T is [tokens=channels, features=window pixels].

v2 structure (vs v1):
- bf16 operands everywhere (PSUM accumulation stays fp32); host pre-casts.
- Zero attention biases let us fold weights on the host:
    scores = T (Wq Wk^T / sqrt(E)) T^T   -> one fused M matrix, no K tensor
    y      = A (T (Wv Wo))               -> one fused N matrix, no separate V/O
- Window-pair batching: QKV-side projections run at N=512 moving dim.
- Phase split: attention for all 8 stripes (Exp table only), then FFN for all
  stripes (Gelu table only) -> 2 ACT table loads instead of ~44, and dense
  back-to-back PE streams that keep the HAM clock-gate warm.
- ys (attention output / residual) kept resident in SBUF in bf16 (8 MB).
- Multi-group bn_stats, batched 2-iteration Newton rsqrt, PSUM evacuations
  balanced across Vector and Scalar engines.
"""

import os
import sys
import numpy as np

for _p in ("/opt/trn_rl_repo", "/root/.axon_site/_ro/trn_rl_repo"):
    if _p not in sys.path and os.path.isdir(_p):
        sys.path.insert(0, _p)

B, S, E, FF = 8, 16384, 256, 1024
H, W, G = 128, 128, 16
Hn, Wn = 8, 8

_CACHE = {}


def _build_fast(ffn_flags):
    """Fused-attention build: requires zero attention biases (bq/bk/bv/bo)."""
    use_b1, use_b2, use_g1, use_g2 = ffn_flags
    import concourse.bacc as bacc
    import concourse.mybir as mybir
    import concourse.tile as tile
    from contextlib import ExitStack

    F32 = mybir.dt.float32
    BF16 = mybir.dt.bfloat16
    I32 = mybir.dt.int32
    AF = mybir.ActivationFunctionType
    OP = mybir.AluOpType

    nc = bacc.Bacc("TRN2", target_bir_lowering=False, debug=False, num_devices=8)

    x_d = nc.dram_tensor("x", [S, E], BF16, kind="ExternalInput")
    m_d = nc.dram_tensor("m", [E, E], BF16, kind="ExternalInput")   # Wq Wk^T / 16
    n_d = nc.dram_tensor("n", [E, E], BF16, kind="ExternalInput")   # Wv Wo
    w1_d = nc.dram_tensor("w1", [E, FF], BF16, kind="ExternalInput")
    w2_d = nc.dram_tensor("w2", [FF, E], BF16, kind="ExternalInput")
    id_d = nc.dram_tensor("ident", [128, 128], BF16, kind="ExternalInput")
    out_d = nc.dram_tensor("out", [S, E], F32, kind="ExternalOutput")
    if use_b1:
        b1_d = nc.dram_tensor("b1", [FF], F32, kind="ExternalInput")
    if use_b2:
        b2_d = nc.dram_tensor("b2", [E], F32, kind="ExternalInput")
    if use_g1:
        g1_d = nc.dram_tensor("g1", [E], F32, kind="ExternalInput")
        be1_d = nc.dram_tensor("be1", [E], F32, kind="ExternalInput")
    if use_g2:
        g2_d = nc.dram_tensor("g2", [E], F32, kind="ExternalInput")
        be2_d = nc.dram_tensor("be2", [E], F32, kind="ExternalInput")

    import concourse.bass as bass

    def bcast_ap(dram, n=256):
        return bass.AP(tensor=dram.ap().tensor, offset=0, ap=[[0, 128], [1, n]])

    X = x_d.ap().rearrange("(c t) e -> c (t e)", t=64)      # [256, 16384]
    OUTV = out_d.ap().rearrange("(c t) e -> c t e", t=64)   # [256, 64, 256]

    with tile.TileContext(nc) as tc:
        with ExitStack() as ctx:
            const = ctx.enter_context(tc.tile_pool(name="const", bufs=1))
            xsp = ctx.enter_context(tc.tile_pool(name="xsp", bufs=4))
            ysp = ctx.enter_context(tc.tile_pool(name="ysp", bufs=16))
            twp = ctx.enter_context(tc.tile_pool(name="twp", bufs=3))
            att = ctx.enter_context(tc.tile_pool(name="att", bufs=3))
            stp = ctx.enter_context(tc.tile_pool(name="stp", bufs=2))
            ffn = ctx.enter_context(tc.tile_pool(name="ffn", bufs=2))
            lnp = ctx.enter_context(tc.tile_pool(name="lnp", bufs=3))
            ztp = ctx.enter_context(tc.tile_pool(name="ztp", bufs=9))
            y2p = ctx.enter_context(tc.tile_pool(name="y2p", bufs=5))
            msc = ctx.enter_context(tc.tile_pool(name="msc", bufs=3))
            outp = ctx.enter_context(tc.tile_pool(name="outp", bufs=4))
            p2 = ctx.enter_context(tc.tile_pool(name="p2", bufs=2, space="PSUM"))
            p1 = ctx.enter_context(tc.tile_pool(name="p1", bufs=2, space="PSUM"))
            pb = ctx.enter_context(tc.tile_pool(name="pb", bufs=2, space="PSUM"))

            ident = const.tile([128, 128], BF16)
            nc.sync.dma_start(out=ident, in_=id_d.ap()[:, :])
            m_t = const.tile([128, 2, 256], BF16)
            nc.sync.dma_start(out=m_t, in_=m_d.ap().rearrange("(ph k) g -> k ph g", k=128))
            n_t = const.tile([128, 2, 256], BF16)
            nc.sync.dma_start(out=n_t, in_=n_d.ap().rearrange("(ph k) g -> k ph g", k=128))
            w1_t = const.tile([128, 2, 1024], BF16)
            nc.sync.dma_start(out=w1_t, in_=w1_d.ap().rearrange("(eh k) f -> k eh f", k=128))
            w2_t = const.tile([128, 8, 256], BF16)
            nc.sync.dma_start(out=w2_t, in_=w2_d.ap().rearrange("(fm k) e -> k fm e", k=128))
            if use_b1:
                b1_t = const.tile([128, 8], F32)
                nc.sync.dma_start(out=b1_t, in_=b1_d.ap().rearrange("(fm p) -> p fm", p=128))
            if use_b2:
                b2_t = const.tile([128, 2], F32)
                nc.sync.dma_start(out=b2_t, in_=b2_d.ap().rearrange("(em p) -> p em", p=128))
            if use_g1:
                g1_bc = const.tile([128, 256], F32)
                nc.sync.dma_start(out=g1_bc, in_=bcast_ap(g1_d))
                be1_bc = const.tile([128, 256], F32)
                nc.sync.dma_start(out=be1_bc, in_=bcast_ap(be1_d))
            if use_g2:
                g2_bc = const.tile([128, 256], F32)
                nc.sync.dma_start(out=g2_bc, in_=bcast_ap(g2_d))
                be2_bc = const.tile([128, 256], F32)
                nc.sync.dma_start(out=be2_bc, in_=bcast_ap(be2_d))

            def newton_rsqrt(var_ap, n, tag, iters=2):
                """rstd = 1/sqrt(var + eps) for a [128, n] strided var AP.

                Bitcast magic-constant seed (~3.4% err) + `iters` Newton
                steps on [128, n]; multiplies routed to GpSimd to offload
                the Vector engine.
                """
                w = msc.tile([128, n], F32, tag=f"nw_w{tag}")
                nc.vector.tensor_scalar(out=w, in0=var_ap, scalar1=1e-5,
                                        scalar2=None, op0=OP.add)
                r = msc.tile([128, n], F32, tag=f"nw_r{tag}")
                nc.vector.tensor_scalar(out=r.bitcast(I32), in0=w.bitcast(I32),
                                        scalar1=1, scalar2=None,
                                        op0=OP.logical_shift_right)
                nc.vector.tensor_scalar(out=r.bitcast(I32), in0=r.bitcast(I32),
                                        scalar1=0xFFFFFFFF, scalar2=None,
                                        op0=OP.bitwise_xor)
                nc.vector.tensor_scalar(out=r.bitcast(I32), in0=r.bitcast(I32),
                                        scalar1=0x5F375A86 + 1, scalar2=None,
                                        op0=OP.add)
                rsq = msc.tile([128, n], F32, tag=f"nw_rsq{tag}")
                u = msc.tile([128, n], F32, tag=f"nw_u{tag}")
                v = msc.tile([128, n], F32, tag=f"nw_v{tag}")
                for _ in range(iters):
                    nc.gpsimd.tensor_mul(rsq, r, r)
                    nc.gpsimd.tensor_mul(u, rsq, w)
                    nc.vector.tensor_scalar(out=v, in0=u, scalar1=-0.5, scalar2=1.5,
                                            op0=OP.mult, op1=OP.add)
                    nc.gpsimd.tensor_mul(r, r, v)
                return r

            ys_all = []  # [hn][ct] -> [128, 2048] bf16 tile

            # ================= Phase A: attention, all stripes =================
            for hn in range(Hn):
                xs_pair = []
                for ct in range(2):
                    t = xsp.tile([128, 2048], BF16, tag="xs")
                    nc.sync.dma_start(
                        out=t, in_=X[ct * 128:(ct + 1) * 128, hn * 2048:(hn + 1) * 2048])
                    xs_pair.append(t)
                ys_pair = [ysp.tile([128, 2048], BF16, tag="ys", name=f"ys{hn}_{i}")
                           for i in range(2)]
                ys_all.append(ys_pair)

                for pk in range(4):  # window pair (2pk, 2pk+1)
                    # gather: per-window contiguous [128ch, 256px] blocks
                    t_sb = twp.tile([128, 2, 2, 256], BF16, tag="tw")
                    for ct in range(2):
                        xv = xs_pair[ct][:, :].rearrange("p (g1 w) -> p g1 w", w=128)
                        for wi in range(2):
                            wn = pk * 2 + wi
                            nc.gpsimd.tensor_copy(
                                t_sb[:, ct, wi, :].rearrange(
                                    "p (g1 g2) -> p g1 g2", g2=16),
                                xv[:, :, wn * 16:(wn + 1) * 16])

                    # tt = T^T per window: [px(ph half), wi, ch(2ct*128)]
                    tt_ps = pb.tile([128, 2, 2, 256], BF16, tag="pb")
                    for ct in range(2):
                        for wi in range(2):
                            for ph in range(2):
                                nc.tensor.transpose(
                                    tt_ps[:, ph, wi, ct * 128:(ct + 1) * 128],
                                    t_sb[:, ct, wi, ph * 128:(ph + 1) * 128],
                                    ident)
                    tt = att.tile([128, 2, 2, 256], BF16, tag="tt")
                    nc.vector.tensor_copy(tt, tt_ps)

                    # u^T = M^T T^T : [g(2 chunks gh), (wi, tok)]
                    u_ps = p2.tile([128, 2, 512], F32, tag="p2")
                    for gh in range(2):
                        for ph in range(2):
                            nc.tensor.matmul(u_ps[:, gh, :],
                                             lhsT=m_t[:, ph, gh * 128:(gh + 1) * 128],
                                             rhs=tt[:, ph, :, :],
                                             start=ph == 0, stop=ph == 1)
                    ut = att.tile([128, 2, 512], BF16, tag="ut")
                    nc.vector.tensor_copy(ut, u_ps)

                    # u2 = T N : [(wi, tok-chunk ct), fo]
                    u2_ps = p2.tile([128, 2, 2, 256], F32, tag="p2")
                    for wi in range(2):
                        for ct in range(2):
                            for ph in range(2):
                                nc.tensor.matmul(
                                    u2_ps[:, wi, ct, :],
                                    lhsT=tt[:, ph, wi, ct * 128:(ct + 1) * 128],
                                    rhs=n_t[:, ph, :],
                                    start=ph == 0, stop=ph == 1)
                    u2 = att.tile([128, 2, 2, 256], BF16, tag="u2")
                    nc.scalar.activation(out=u2, in_=u2_ps, func=AF.Copy)

                    # scores + exp (unnormalized), denominator accumulated
                    den = stp.tile([128, 4], F32, tag="den")
                    aa = att.tile([128, 2, 2, 256], BF16, tag="aa")
                    for wi in range(2):
                        s_ps = p1.tile([128, 2, 256], F32, tag="p1")
                        for th in range(2):
                            for gh in range(2):
                                nc.tensor.matmul(
                                    s_ps[:, th, :],
                                    lhsT=ut[:, gh, wi * 256 + th * 128:
                                            wi * 256 + (th + 1) * 128],
                                    rhs=tt[:, gh, wi, :],
                                    start=gh == 0, stop=gh == 1)
                        for th in range(2):
                            nc.scalar.activation(
                                out=aa[:, wi, th, :], in_=s_ps[:, th, :],
                                func=AF.Exp,
                                accum_out=den[:, wi * 2 + th:wi * 2 + th + 1])
                    rec = stp.tile([128, 4], F32, tag="rec")
                    nc.vector.reciprocal(rec, den)

                    # at = A^T (unnormalized): [(wi), k-chunk t2h, q]
                    at_ps = pb.tile([128, 2, 2, 256], BF16, tag="pb")
                    for wi in range(2):
                        for th in range(2):
                            for t2h in range(2):
                                nc.tensor.transpose(
                                    at_ps[:, wi, t2h, th * 128:(th + 1) * 128],
                                    aa[:, wi, th, t2h * 128:(t2h + 1) * 128],
                                    ident)
                    at = att.tile([128, 2, 2, 256], BF16, tag="at")
                    nc.vector.tensor_copy(at, at_ps)

                    # y_window = A u2, scaled by 1/den at evacuation
                    for wi in range(2):
                        o_ps = p1.tile([128, 2, 256], F32, tag="p1")
                        for th in range(2):
                            for t2h in range(2):
                                nc.tensor.matmul(
                                    o_ps[:, th, :],
                                    lhsT=at[:, wi, t2h, th * 128:(th + 1) * 128],
                                    rhs=u2[:, wi, t2h, :],
                                    start=t2h == 0, stop=t2h == 1)
                        wn = pk * 2 + wi
                        for th in range(2):
                            ys_sl = ys_pair[th][:, :].rearrange(
                                "p (g1 w) -> p g1 w", w=128)[:, :, wn * 16:(wn + 1) * 16]
                            if th == 0:
                                nc.vector.tensor_scalar(
                                    out=ys_sl,
                                    in0=o_ps[:, th, :].rearrange(
                                        "p (a b) -> p a b", b=16),
                                    scalar1=rec[:, wi * 2 + th:wi * 2 + th + 1],
                                    scalar2=None, op0=OP.mult)
                            else:
                                nc.scalar.activation(
                                    out=ys_sl,
                                    in_=o_ps[:, th, :].rearrange(
                                        "p (a b) -> p a b", b=16),
                                    func=AF.Copy,
                                    scale=rec[:, wi * 2 + th:wi * 2 + th + 1])

            # ================= Phase B: FFN + LNs, all stripes =================
            # Software-pipelined at stripe level: the matmul pass of stripe
            # hn runs before the LN pass of stripe hn-1, so the PE never
            # waits behind the serial LN chain for its yt/zt evacuations.
            def mm_pass(hn):
                ys_pair = ys_all[hn]
                z_tiles = []
                mvs1 = msc.tile([128, 16, 2], F32, tag="mvs1", name=f"mvs1_{hn}")
                for nb in range(4):
                    yt = ffn.tile([128, 2, 512], BF16, tag="yt")
                    yt_ps = pb.tile([128, 2, 512], BF16, tag="pb")
                    for eh in range(2):
                        for pos, (ct, j) in enumerate(
                                (q // 8, q % 8) for q in range(nb * 4, nb * 4 + 4)):
                            nc.tensor.transpose(
                                yt_ps[:, eh, pos * 128:(pos + 1) * 128],
                                ys_pair[ct][:, j * 256 + eh * 128:
                                            j * 256 + (eh + 1) * 128],
                                ident)
                    nc.vector.tensor_copy(yt, yt_ps)

                    hh = ffn.tile([128, 8, 512], BF16, tag="hh")
                    for fp in range(4):
                        h_ps = p2.tile([128, 2, 512], F32, tag="p2")
                        for i in range(2):
                            fm = fp * 2 + i
                            for eh in range(2):
                                nc.tensor.matmul(h_ps[:, i, :],
                                                 lhsT=w1_t[:, eh, fm * 128:(fm + 1) * 128],
                                                 rhs=yt[:, eh, :],
                                                 start=eh == 0, stop=eh == 1)
                        if use_b1:
                            for i in range(2):
                                fm = fp * 2 + i
                                nc.scalar.activation(out=hh[:, fm, :], in_=h_ps[:, i, :],
                                                     func=AF.Gelu,
                                                     bias=b1_t[:, fm:fm + 1])
                        else:
                            nc.scalar.activation(out=hh[:, fp * 2:(fp + 1) * 2, :],
                                                 in_=h_ps, func=AF.Gelu)

                    ft = ffn.tile([128, 2, 512], BF16, tag="ft")
                    for em in range(2):
                        f_ps = p1.tile([128, 512], F32, tag="p1")
                        for fm in range(8):
                            nc.tensor.matmul(f_ps,
                                             lhsT=w2_t[:, fm, em * 128:(em + 1) * 128],
                                             rhs=hh[:, fm, :],
                                             start=fm == 0, stop=fm == 7)
                        if use_b2:
                            nc.scalar.activation(out=ft[:, em, :], in_=f_ps,
                                                 func=AF.Identity,
                                                 bias=b2_t[:, em:em + 1])
                        else:
                            nc.scalar.activation(out=ft[:, em, :], in_=f_ps,
                                                 func=AF.Copy)

                    z_ps = pb.tile([128, 4, 256], BF16, tag="pb")
                    for pos in range(4):
                        for em in range(2):
                            nc.tensor.transpose(
                                z_ps[:, pos, em * 128:(em + 1) * 128],
                                ft[:, em, pos * 128:(pos + 1) * 128],
                                ident)
                    zt = ztp.tile([128, 4, 256], BF16, tag="zt",
                                  name=f"zt{hn}_{nb}")
                    nc.vector.tensor_copy(zt, z_ps)
                    z_tiles.append(zt)

                    bst1 = msc.tile([128, 6], F32, tag="bst1")
                    for pos in range(4):
                        nc.vector.bn_stats(out=bst1, in_=zt[:, pos, :])
                        nc.vector.bn_aggr(out=mvs1[:, nb * 4 + pos, :], in_=bst1)
                return z_tiles, mvs1

            def ln_pass(hn, z_tiles, mvs1):
                ys_pair = ys_all[hn]
                rs1 = newton_rsqrt(mvs1[:, :, 1], 16, "a", iters=2)
                # per-partition bias for the ACT-side LN1 apply: -m1*rs1
                mb1 = msc.tile([128, 16], F32, tag="mb1")
                nc.gpsimd.tensor_mul(mb1, mvs1[:, :, 0], rs1)
                nc.vector.tensor_scalar(out=mb1, in0=mb1, scalar1=-1.0,
                                        scalar2=None, op0=OP.mult)

                mvs2 = msc.tile([128, 16, 2], F32, tag="mvs2")
                y2_tiles = []
                for nb in range(4):
                    chunks = [(q // 8, q % 8) for q in range(nb * 4, nb * 4 + 4)]
                    zt = z_tiles[nb]
                    y2 = y2p.tile([128, 4, 256], BF16, tag="y2",
                                  name=f"y2{hn}_{nb}")
                    bst2 = msc.tile([128, 6], F32, tag="bst2")
                    for pos, (ct, j) in enumerate(chunks):
                        k = nb * 4 + pos
                        ln1 = lnp.tile([128, 256], BF16, tag="ln1")
                        nc.scalar.activation(
                            out=ln1, in_=zt[:, pos, :], func=AF.Identity,
                            scale=rs1[:, k:k + 1], bias=mb1[:, k:k + 1])
                        if use_g1:
                            nc.gpsimd.tensor_mul(ln1, ln1, g1_bc)
                            nc.gpsimd.tensor_add(ln1, ln1, be1_bc)
                        nc.gpsimd.tensor_add(
                            y2[:, pos, :], ln1,
                            ys_pair[ct][:, j * 256:(j + 1) * 256])
                        nc.vector.bn_stats(out=bst2, in_=y2[:, pos, :])
                        nc.vector.bn_aggr(out=mvs2[:, k, :], in_=bst2)
                    y2_tiles.append(y2)

                rs2 = newton_rsqrt(mvs2[:, :, 1], 16, "b", iters=2)
                # out = y2 + ln2 = y2*(1+rs2) - m2*rs2  (one fused op per chunk)
                a2 = msc.tile([128, 16], F32, tag="a2")
                nc.vector.tensor_scalar(out=a2, in0=rs2, scalar1=1.0,
                                        scalar2=None, op0=OP.add)
                b2m = msc.tile([128, 16], F32, tag="b2m")
                nc.gpsimd.tensor_mul(b2m, mvs2[:, :, 0], rs2)

                for nb in range(4):
                    chunks = [(q // 8, q % 8) for q in range(nb * 4, nb * 4 + 4)]
                    y2 = y2_tiles[nb]
                    for pos, (ct, j) in enumerate(chunks):
                        k = nb * 4 + pos
                        if use_g2:
                            ln2 = lnp.tile([128, 256], BF16, tag="ln2")
                            nc.vector.tensor_scalar(
                                out=ln2, in0=y2[:, pos, :],
                                scalar1=mvs2[:, k, 0:1], scalar2=rs2[:, k:k + 1],
                                op0=OP.subtract, op1=OP.mult)
                            nc.gpsimd.tensor_mul(ln2, ln2, g2_bc)
                            nc.gpsimd.tensor_add(ln2, ln2, be2_bc)
                            outt = outp.tile([128, 256], F32, tag="outt")
                            nc.gpsimd.tensor_add(outt, ln2, y2[:, pos, :])
                        else:
                            outt = outp.tile([128, 256], F32, tag="outt")
                            nc.vector.tensor_scalar(
                                out=outt, in0=y2[:, pos, :],
                                scalar1=a2[:, k:k + 1], scalar2=b2m[:, k:k + 1],
                                op0=OP.mult, op1=OP.subtract)
                        nc.sync.dma_start(
                            out=OUTV[ct * 128:(ct + 1) * 128, hn * 8 + j, :],
                            in_=outt)

            pending = None
            for hn in range(Hn):
                cur = mm_pass(hn)
                if pending is not None:
                    ln_pass(hn - 1, *pending)
                pending = cur
            ln_pass(Hn - 1, *pending)

    nc.compile()
    return nc


def _round_f32r(x):
    u = np.ascontiguousarray(x, np.float32).view(np.uint32)
    return ((u + np.uint32(0x800)) & np.uint32(0xFFFFF000)).view(np.float32)


def _build_v1(flags):
    """Unfused fallback (handles attention biases); f32r, per-window."""
    use_bqk, use_bv, use_bo, use_b1, use_b2, use_g1, use_g2 = flags
    import concourse.bacc as bacc
    import concourse.mybir as mybir
    import concourse.tile as tile
    from contextlib import ExitStack

    F32 = mybir.dt.float32
    F32R = mybir.dt.float32r
    I32 = mybir.dt.int32
    AF = mybir.ActivationFunctionType
    OP = mybir.AluOpType

    nc = bacc.Bacc("TRN2", target_bir_lowering=False, debug=False, num_devices=8)

    x_d = nc.dram_tensor("x", [S, E], F32R, kind="ExternalInput")
    wq_d = nc.dram_tensor("wq", [E, E], F32R, kind="ExternalInput")
    wk_d = nc.dram_tensor("wk", [E, E], F32R, kind="ExternalInput")
    wv_d = nc.dram_tensor("wv", [E, E], F32R, kind="ExternalInput")
    wo_d = nc.dram_tensor("wo", [E, E], F32R, kind="ExternalInput")
    w1_d = nc.dram_tensor("w1", [E, FF], F32R, kind="ExternalInput")
    w2_d = nc.dram_tensor("w2", [FF, E], F32R, kind="ExternalInput")
    id_d = nc.dram_tensor("ident", [128, 128], F32R, kind="ExternalInput")
    out_d = nc.dram_tensor("out", [S, E], F32, kind="ExternalOutput")
    if use_bqk:
        bq_d = nc.dram_tensor("bq", [E], F32, kind="ExternalInput")
        bk_d = nc.dram_tensor("bk", [E], F32, kind="ExternalInput")
    if use_bv:
        bv_d = nc.dram_tensor("bv", [E], F32, kind="ExternalInput")
    if use_bo:
        bo_d = nc.dram_tensor("bo", [E], F32, kind="ExternalInput")
    if use_b1:
        b1_d = nc.dram_tensor("b1", [FF], F32, kind="ExternalInput")
    if use_b2:
        b2_d = nc.dram_tensor("b2", [E], F32, kind="ExternalInput")
    if use_g1:
        g1_d = nc.dram_tensor("g1", [E], F32, kind="ExternalInput")
        be1_d = nc.dram_tensor("be1", [E], F32, kind="ExternalInput")
    if use_g2:
        g2_d = nc.dram_tensor("g2", [E], F32, kind="ExternalInput")
        be2_d = nc.dram_tensor("be2", [E], F32, kind="ExternalInput")

    import concourse.bass as bass

    def bcast_ap(dram, n=256):
        return bass.AP(tensor=dram.ap().tensor, offset=0, ap=[[0, 128], [1, n]])

    X = x_d.ap().rearrange("(c t) e -> c (t e)", t=64)
    OUTV = out_d.ap().rearrange("(c t) e -> c t e", t=64)

    with tile.TileContext(nc) as tc:
        with ExitStack() as ctx:
            const = ctx.enter_context(tc.tile_pool(name="const", bufs=1))
            xsp = ctx.enter_context(tc.tile_pool(name="xsp", bufs=4))
            ysp = ctx.enter_context(tc.tile_pool(name="ysp", bufs=4))
            twp = ctx.enter_context(tc.tile_pool(name="twp", bufs=3))
            att = ctx.enter_context(tc.tile_pool(name="att", bufs=3))
            stp = ctx.enter_context(tc.tile_pool(name="stp", bufs=4))
            ffn = ctx.enter_context(tc.tile_pool(name="ffn", bufs=2))
            lnp = ctx.enter_context(tc.tile_pool(name="lnp", bufs=4))
            msc = ctx.enter_context(tc.tile_pool(name="msc", bufs=4))
            pA = ctx.enter_context(tc.tile_pool(name="pA", bufs=3, space="PSUM"))
            pH = ctx.enter_context(tc.tile_pool(name="pH", bufs=1, space="PSUM"))
            pF = ctx.enter_context(tc.tile_pool(name="pF", bufs=3, space="PSUM"))

            ident = const.tile([128, 128], F32R)
            nc.sync.dma_start(out=ident, in_=id_d.ap()[:, :])
            wq_t = const.tile([128, 2, 256], F32R)
            wk_t = const.tile([128, 2, 256], F32R)
            wv_t = const.tile([128, 2, 256], F32R)
            wo_t = const.tile([128, 2, 256], F32R)
            for t, d in ((wq_t, wq_d), (wk_t, wk_d), (wv_t, wv_d), (wo_t, wo_d)):
                nc.sync.dma_start(out=t, in_=d.ap().rearrange("(eh k) f -> k eh f", k=128))
            w1_t = const.tile([128, 2, 1024], F32R)
            nc.sync.dma_start(out=w1_t, in_=w1_d.ap().rearrange("(eh k) f -> k eh f", k=128))
            w2_t = const.tile([128, 8, 256], F32R)
            nc.sync.dma_start(out=w2_t, in_=w2_d.ap().rearrange("(fm k) e -> k fm e", k=128))
            if use_bqk:
                bq_t = const.tile([128, 2], F32)
                nc.sync.dma_start(out=bq_t, in_=bq_d.ap().rearrange("(fh p) -> p fh", p=128))
                bk_t = const.tile([128, 2], F32)
                nc.sync.dma_start(out=bk_t, in_=bk_d.ap().rearrange("(fh p) -> p fh", p=128))
            if use_bv:
                bv_bc = const.tile([128, 2, 256], F32)
                nc.sync.dma_start(
                    out=bv_bc,
                    in_=bass.AP(tensor=bv_d.ap().tensor, offset=0,
                                ap=[[0, 128], [0, 2], [1, 256]]))
            if use_bo:
                bo_st = const.tile([128, 2048], F32)
                nc.sync.dma_start(
                    out=bo_st.rearrange("p (g1 wn g2) -> p g1 wn g2", wn=8, g2=16),
                    in_=bass.AP(tensor=bo_d.ap().tensor, offset=0,
                                ap=[[0, 128], [16, 16], [0, 8], [1, 16]]))
            if use_b1:
                b1_t = const.tile([128, 8], F32)
                nc.sync.dma_start(out=b1_t, in_=b1_d.ap().rearrange("(fm p) -> p fm", p=128))
            if use_b2:
                b2_t = const.tile([128, 2], F32)
                nc.sync.dma_start(out=b2_t, in_=b2_d.ap().rearrange("(em p) -> p em", p=128))
            if use_g1:
                g1_bc = const.tile([128, 256], F32)
                nc.sync.dma_start(out=g1_bc, in_=bcast_ap(g1_d))
                be1_bc = const.tile([128, 256], F32)
                nc.sync.dma_start(out=be1_bc, in_=bcast_ap(be1_d))
            if use_g2:
                g2_bc = const.tile([128, 256], F32)
                nc.sync.dma_start(out=g2_bc, in_=bcast_ap(g2_d))
                be2_bc = const.tile([128, 256], F32)
                nc.sync.dma_start(out=be2_bc, in_=bcast_ap(be2_d))

            def newton_rsqrt(var_ap, n):
                w = msc.tile([128, n], F32, tag="nw_w")
                nc.vector.tensor_scalar(out=w, in0=var_ap, scalar1=1e-5,
                                        scalar2=None, op0=OP.add)
                r = msc.tile([128, n], F32, tag="nw_r")
                nc.vector.tensor_scalar(out=r.bitcast(I32), in0=w.bitcast(I32),
                                        scalar1=1, scalar2=None,
                                        op0=OP.logical_shift_right)
                nc.vector.tensor_scalar(out=r.bitcast(I32), in0=r.bitcast(I32),
                                        scalar1=0xFFFFFFFF, scalar2=None,
                                        op0=OP.bitwise_xor)
                nc.vector.tensor_scalar(out=r.bitcast(I32), in0=r.bitcast(I32),
                                        scalar1=0x5F375A86 + 1, scalar2=None,
                                        op0=OP.add)
                rsq = msc.tile([128, n], F32, tag="nw_rsq")
                u = msc.tile([128, n], F32, tag="nw_u")
                v = msc.tile([128, n], F32, tag="nw_v")
                for _ in range(3):
                    nc.vector.tensor_mul(rsq, r, r)
                    nc.vector.tensor_mul(u, rsq, w)
                    nc.vector.tensor_scalar(out=v, in0=u, scalar1=-0.5, scalar2=1.5,
                                            op0=OP.mult, op1=OP.add)
                    nc.vector.tensor_mul(r, r, v)
                return r

            for hn in range(Hn):
                xs_pair = []
                for ct in range(2):
                    t = xsp.tile([128, 2048], F32R, tag="xs")
                    nc.sync.dma_start(
                        out=t, in_=X[ct * 128:(ct + 1) * 128, hn * 2048:(hn + 1) * 2048])
                    xs_pair.append(t)
                ys_pair = [ysp.tile([128, 2048], F32R, tag="ys", name=f"ys{hn}_{i}")
                           for i in range(2)]

                for wn in range(Wn):
                    t_sb = twp.tile([128, 2, 256], F32R, tag="tw")
                    for ct in range(2):
                        xv = xs_pair[ct][:, :].rearrange("p (g1 w) -> p g1 w", w=128)
                        nc.gpsimd.tensor_copy(
                            t_sb[:, ct, :].rearrange("p (g1 g2) -> p g1 g2", g2=16),
                            xv[:, :, wn * 16:(wn + 1) * 16])
                    tt_ps = pA.tile([128, 2, 256], F32, tag="pA")
                    for eh in range(2):
                        for ct in range(2):
                            nc.tensor.transpose(
                                tt_ps[:, eh, ct * 128:(ct + 1) * 128].bitcast(F32R),
                                t_sb[:, ct, eh * 128:(eh + 1) * 128], ident)
                    tt = att.tile([128, 2, 256], F32R, tag="tt")
                    nc.vector.tensor_copy(tt, tt_ps)

                    qt_ps = pA.tile([128, 2, 256], F32, tag="pA")
                    for fh in range(2):
                        for eh in range(2):
                            nc.tensor.matmul(qt_ps[:, fh, :],
                                             lhsT=wq_t[:, eh, fh * 128:(fh + 1) * 128],
                                             rhs=tt[:, eh, :],
                                             start=eh == 0, stop=eh == 1)
                    qt = att.tile([128, 2, 256], F32R, tag="qt")
                    if use_bqk:
                        for fh in range(2):
                            nc.scalar.activation(out=qt[:, fh, :], in_=qt_ps[:, fh, :],
                                                 func=AF.Identity,
                                                 bias=bq_t[:, fh:fh + 1])
                    else:
                        nc.vector.tensor_copy(qt, qt_ps)

                    kt_ps = pA.tile([128, 2, 256], F32, tag="pA")
                    for fh in range(2):
                        for eh in range(2):
                            nc.tensor.matmul(kt_ps[:, fh, :],
                                             lhsT=wk_t[:, eh, fh * 128:(fh + 1) * 128],
                                             rhs=tt[:, eh, :],
                                             start=eh == 0, stop=eh == 1)
                    kt = att.tile([128, 2, 256], F32R, tag="kt")
                    if use_bqk:
                        for fh in range(2):
                            nc.scalar.activation(out=kt[:, fh, :], in_=kt_ps[:, fh, :],
                                                 func=AF.Identity,
                                                 bias=bk_t[:, fh:fh + 1])
                    else:
                        nc.vector.tensor_copy(kt, kt_ps)

                    v_ps = pA.tile([128, 2, 256], F32, tag="pA")
                    for ch in range(2):
                        for eh in range(2):
                            nc.tensor.matmul(v_ps[:, ch, :],
                                             lhsT=tt[:, eh, ch * 128:(ch + 1) * 128],
                                             rhs=wv_t[:, eh, :],
                                             start=eh == 0, stop=eh == 1)
                    vv = att.tile([128, 2, 256], F32R, tag="vv")
                    if use_bv:
                        nc.vector.tensor_add(vv, v_ps, bv_bc)
                    else:
                        nc.scalar.activation(out=vv, in_=v_ps, func=AF.Copy)

                    s_ps = pA.tile([128, 2, 256], F32, tag="pA")
                    for th in range(2):
                        for fh in range(2):
                            nc.tensor.matmul(s_ps[:, th, :],
                                             lhsT=qt[:, fh, th * 128:(th + 1) * 128],
                                             rhs=kt[:, fh, :],
                                             start=fh == 0, stop=fh == 1)
                    aa = att.tile([128, 2, 256], F32R, tag="aa")
                    den = stp.tile([128, 2], F32, tag="den")
                    for th in range(2):
                        nc.scalar.activation(out=aa[:, th, :], in_=s_ps[:, th, :],
                                             func=AF.Exp,
                                             accum_out=den[:, th:th + 1])
                    rec = stp.tile([128, 2], F32, tag="rec")
                    nc.vector.reciprocal(rec, den)

                    at_ps = pA.tile([128, 2, 256], F32, tag="pA")
                    for t2h in range(2):
                        for th in range(2):
                            nc.tensor.transpose(
                                at_ps[:, t2h, th * 128:(th + 1) * 128].bitcast(F32R),
                                aa[:, th, t2h * 128:(t2h + 1) * 128], ident)
                    at = att.tile([128, 2, 256], F32R, tag="at")
                    nc.scalar.activation(out=at, in_=at_ps, func=AF.Copy)

                    ot_ps = pA.tile([128, 2, 256], F32, tag="pA")
                    for fh in range(2):
                        for t2h in range(2):
                            nc.tensor.matmul(ot_ps[:, fh, :],
                                             lhsT=vv[:, t2h, fh * 128:(fh + 1) * 128],
                                             rhs=at[:, t2h, :],
                                             start=t2h == 0, stop=t2h == 1)
                    ot = att.tile([128, 2, 256], F32R, tag="ot")
                    nc.scalar.activation(out=ot, in_=ot_ps, func=AF.Copy)

                    o2_ps = pA.tile([128, 2, 256], F32, tag="pA")
                    for th in range(2):
                        for fh in range(2):
                            nc.tensor.matmul(o2_ps[:, th, :],
                                             lhsT=ot[:, fh, th * 128:(th + 1) * 128],
                                             rhs=wo_t[:, fh, :],
                                             start=fh == 0, stop=fh == 1)
                    for th in range(2):
                        ys_sl = ys_pair[th][:, :].rearrange(
                            "p (g1 w) -> p g1 w", w=128)[:, :, wn * 16:(wn + 1) * 16]
                        nc.vector.tensor_scalar(
                            out=ys_sl,
                            in0=o2_ps[:, th, :].rearrange("p (a b) -> p a b", b=16),
                            scalar1=rec[:, th:th + 1], scalar2=None, op0=OP.mult)

                if use_bo:
                    for ct in range(2):
                        nc.gpsimd.tensor_add(ys_pair[ct], ys_pair[ct].bitcast(F32), bo_st)

                for nb in range(4):
                    chunks = [(q // 8, q % 8) for q in range(nb * 4, nb * 4 + 4)]
                    yt = ffn.tile([128, 2, 512], F32R, tag="yt")
                    for eh in range(2):
                        yt_ps = pA.tile([128, 512], F32, tag="pA")
                        for pos, (ct, j) in enumerate(chunks):
                            nc.tensor.transpose(
                                yt_ps[:, pos * 128:(pos + 1) * 128].bitcast(F32R),
                                ys_pair[ct][:, j * 256 + eh * 128: j * 256 + (eh + 1) * 128],
                                ident)
                        nc.vector.tensor_copy(yt[:, eh, :], yt_ps)

                    hh = ffn.tile([128, 8, 512], F32R, tag="hh")
                    for fp in range(4):
                        h_ps = pH.tile([128, 2, 512], F32, tag="pH")
                        for i in range(2):
                            fm = fp * 2 + i
                            for eh in range(2):
                                nc.tensor.matmul(h_ps[:, i, :],
                                                 lhsT=w1_t[:, eh, fm * 128:(fm + 1) * 128],
                                                 rhs=yt[:, eh, :],
                                                 start=eh == 0, stop=eh == 1)
                        if use_b1:
                            for i in range(2):
                                fm = fp * 2 + i
                                nc.scalar.activation(out=hh[:, fm, :], in_=h_ps[:, i, :],
                                                     func=AF.Gelu,
                                                     bias=b1_t[:, fm:fm + 1])
                        else:
                            nc.scalar.activation(out=hh[:, fp * 2:(fp + 1) * 2, :],
                                                 in_=h_ps, func=AF.Gelu)

                    ft = ffn.tile([128, 2, 512], F32R, tag="ft")
                    for em in range(2):
                        f_ps = pF.tile([128, 512], F32, tag="pF")
                        for fm in range(8):
                            nc.tensor.matmul(f_ps,
                                             lhsT=w2_t[:, fm, em * 128:(em + 1) * 128],
                                             rhs=hh[:, fm, :],
                                             start=fm == 0, stop=fm == 7)
                        if use_b2:
                            nc.scalar.activation(out=ft[:, em, :], in_=f_ps,
                                                 func=AF.Identity,
                                                 bias=b2_t[:, em:em + 1])
                        else:
                            nc.vector.tensor_copy(ft[:, em, :], f_ps)

                    z_ps = []
                    for pp in range(2):
                        zp = pF.tile([128, 2, 256], F32, tag="pF")
                        for i in range(2):
                            pos = pp * 2 + i
                            for em in range(2):
                                nc.tensor.transpose(
                                    zp[:, i, em * 128:(em + 1) * 128].bitcast(F32R),
                                    ft[:, em, pos * 128:(pos + 1) * 128], ident)
                        z_ps.append(zp)

                    mvs1 = msc.tile([128, 4, 2], F32, tag="mvs1")
                    for pos in range(4):
                        bst = msc.tile([128, 6], F32, tag="bst")
                        nc.vector.bn_stats(out=bst, in_=z_ps[pos // 2][:, pos % 2, :])
                        nc.vector.bn_aggr(out=mvs1[:, pos, :], in_=bst)
                    rs1 = newton_rsqrt(mvs1[:, :, 1], 4)

                    y2s = []
                    mvs2 = msc.tile([128, 4, 2], F32, tag="mvs2")
                    for pos, (ct, j) in enumerate(chunks):
                        ln1 = lnp.tile([128, 256], F32, tag="ln1")
                        nc.vector.tensor_scalar(
                            out=ln1, in0=z_ps[pos // 2][:, pos % 2, :],
                            scalar1=mvs1[:, pos, 0:1], scalar2=rs1[:, pos:pos + 1],
                            op0=OP.subtract, op1=OP.mult)
                        if use_g1:
                            nc.gpsimd.tensor_mul(ln1, ln1, g1_bc)
                            nc.gpsimd.tensor_add(ln1, ln1, be1_bc)
                        y2 = lnp.tile([128, 256], F32, tag="y2")
                        nc.gpsimd.tensor_add(
                            y2, ln1,
                            ys_pair[ct][:, j * 256:(j + 1) * 256].bitcast(F32))
                        y2s.append(y2)
                        bst = msc.tile([128, 6], F32, tag="bst")
                        nc.vector.bn_stats(out=bst, in_=y2)
                        nc.vector.bn_aggr(out=mvs2[:, pos, :], in_=bst)
                    rs2 = newton_rsqrt(mvs2[:, :, 1], 4)

                    for pos, (ct, j) in enumerate(chunks):
                        ln2 = lnp.tile([128, 256], F32, tag="ln2")
                        nc.vector.tensor_scalar(
                            out=ln2, in0=y2s[pos],
                            scalar1=mvs2[:, pos, 0:1], scalar2=rs2[:, pos:pos + 1],
                            op0=OP.subtract, op1=OP.mult)
                        if use_g2:
                            nc.gpsimd.tensor_mul(ln2, ln2, g2_bc)
                            nc.gpsimd.tensor_add(ln2, ln2, be2_bc)
                        outt = lnp.tile([128, 256], F32, tag="outt")
                        nc.gpsimd.tensor_add(outt, ln2, y2s[pos])
                        nc.sync.dma_start(
                            out=OUTV[ct * 128:(ct + 1) * 128, hn * 8 + j, :],
                            in_=outt)

    nc.compile()
    return nc


def _inputs_v1(flags, x, Wq, Wk, Wv, Wo, W1, W2, bq, bk, bv, bo, b1, b2,
               g1, be1, g2, be2):
    scale = 1.0 / np.sqrt(np.float32(E))
    base = {
        "wq": _round_f32r(Wq * scale),
        "wk": _round_f32r(Wk),
        "wv": _round_f32r(Wv),
        "wo": _round_f32r(Wo),
        "w1": _round_f32r(W1),
        "w2": _round_f32r(W2),
        "ident": np.eye(128, dtype=np.float32),
    }
    use_bqk, use_bv, use_bo, use_b1, use_b2, use_g1, use_g2 = flags
    if use_bqk:
        base["bq"] = bq * scale
        base["bk"] = bk
    if use_bv:
        base["bv"] = bv
    if use_bo:
        base["bo"] = bo
    if use_b1:
        base["b1"] = b1
    if use_b2:
        base["b2"] = b2
    if use_g1:
        base["g1"] = g1
        base["be1"] = be1
    if use_g2:
        base["g2"] = g2
        base["be2"] = be2
    return [dict(base, x=_round_f32r(x[b])) for b in range(B)]


def _build(flags):
    use_bqk, use_bv, use_bo, use_b1, use_b2, use_g1, use_g2 = flags
    if not (use_bqk or use_bv or use_bo):
        return _build_fast((use_b1, use_b2, use_g1, use_g2)), True
    return _build_v1(flags), False


def _get_program(flags):
    if flags not in _CACHE:
        _CACHE[flags] = _build(flags)
    return _CACHE[flags]


def kernel(**inputs):
    import ml_dtypes

    bf16 = ml_dtypes.bfloat16
    x = np.asarray(inputs["x"], np.float32)
    Wq = np.asarray(inputs["Wq"], np.float32)
    Wk = np.asarray(inputs["Wk"], np.float32)
    Wv = np.asarray(inputs["Wv"], np.float32)
    Wo = np.asarray(inputs["Wo"], np.float32)
    W1 = np.asarray(inputs["W1"], np.float32)
    W2 = np.asarray(inputs["W2"], np.float32)
    bq = np.asarray(inputs["bq"], np.float32)
    bk = np.asarray(inputs["bk"], np.float32)
    bv = np.asarray(inputs["bv"], np.float32)
    bo = np.asarray(inputs["bo"], np.float32)
    b1 = np.asarray(inputs["b1"], np.float32)
    b2 = np.asarray(inputs["b2"], np.float32)
    g1 = np.asarray(inputs["g1"], np.float32)
    be1 = np.asarray(inputs["be1"], np.float32)
    g2 = np.asarray(inputs["g2"], np.float32)
    be2 = np.asarray(inputs["be2"], np.float32)

    flags = (
        bool(bq.any() or bk.any()),
        bool(bv.any()),
        bool(bo.any()),
        bool(b1.any()),
        bool(b2.any()),
        bool((g1 != 1.0).any() or be1.any()),
        bool((g2 != 1.0).any() or be2.any()),
    )
    nc, fast = _get_program(flags)

    scale = 1.0 / np.sqrt(np.float32(E))
    if fast:
        use_bqk, use_bv, use_bo, use_b1, use_b2, use_g1, use_g2 = flags
        M = (Wq @ Wk.T) * scale
        N = Wv @ Wo
        base = {
            "m": M.astype(bf16),
            "n": N.astype(bf16),
            "w1": W1.astype(bf16),
            "w2": W2.astype(bf16),
            "ident": np.eye(128, dtype=np.float32).astype(bf16),
        }
        if use_b1:
            base["b1"] = b1
        if use_b2:
            base["b2"] = b2
        if use_g1:
            base["g1"] = g1
            base["be1"] = be1
        if use_g2:
            base["g2"] = g2
            base["be2"] = be2
        in_maps = [dict(base, x=x[b].astype(bf16)) for b in range(B)]
    else:
        in_maps = _inputs_v1(flags, x, Wq, Wk, Wv, Wo, W1, W2, bq, bk, bv, bo,
                             b1, b2, g1, be1, g2, be2)

    from concourse.bass_utils import run_bass_kernel_spmd

    res = run_bass_kernel_spmd(nc, in_maps, list(range(B)))
    kernel.last_exec_time_ns = res.exec_time_ns
    kernel.last_trace = res.instructions_and_trace
    kernel.last_profile_json = res.profile_json
    return np.stack([r["out"] for r in res.results], axis=0)
